# revision 57
# baseline (speedup 1.0000x reference)
"""BalSCL (balanced supervised contrastive loss) for Trainium2, 8 NeuronCores.

v3: all-fp8e5 exp tiles + K=256 DoubleRow E1 reduce, mixed PSUM ring.
TimelineSim 45623 ns (v2 bf16 baseline: 50189 ns), rel err ~4e-4.

Math (same restructure as v2): with tgt = [targets, targets, arange(C)],
feats = [view0, view1, centers] (L2-normalized, fp8e4m3-rounded on host),
the device computes per-class exp sums
    E1[k, i] = sum_{j: tgt_j = k} e5m2(e^{10 * S_ij}),  S = feats . feats[anchors]^T
and the host (float64) finishes:
    A_i = sum_k E1[k,i]/cnt[k] + (1/(cnt-1) - 1/cnt) E1[t_i, i] - e_ii/(cnt-1)
    loss_i = log(A_i) - 10 * (f_i . G[t_i] - S_ii) / (cnt[t_i]-1)

Device structure per core (1024 anchors = 2 blocks of 512 columns):
  - S matmul: fp8e4 DoubleRow ([64, 2, 128] lhsT packing of D=128) ->
    107 ns per 128-row j-chunk (0.5 cyc/row at 2.4 GHz).
  - exp -> fp8e5 tiles on ACT (table exp, e5m2 out dtype) and DVE
    (Schraudolph bit trick: i8 = round(S*C8 + B8), bitcast e5m2; both
    replicated bit-exactly on the host for the diagonal).  GPSIMD cannot
    read PSUM (BIR verifier), so only these two engines can consume S;
    their ~37 us/engine crossing of the S matrix IS the wall.  e5m2's
    32-binade range holds everything incl. the e^10 diagonal (e4m3 would
    overflow at exp(10*0.613) = 458 > 448).
  - reduce: ONE fp8e5 DoubleRow matmul per chunk pair (K=256: lhsT =
    onehot pair [128, 2, 112], rhs = e tile [128, 2, 512]) -> 107 ns per
    pair, 4x cheaper than v2's bf16 per-chunk reduce.  Classes padded
    100->112 for the 16B lhsT sub-row alignment restriction
    (s3_lw_dual_fp8).  Single-chunk units use plain (non-DR) fp8e5
    matmuls.  PE totals ~21.5 us - far off the critical path.
  - PSUM (8 banks): 3 full S tiles (2 banks each) + one half tile 'H'
    + a single-buffered E1 accumulator.  The 3-deep full-tile ring alone
    caps throughput (tile recycle latency exp-end -> S -> exp-start is
    ~0.8 us); the H-tile singles woven into each 14-chunk cycle relieve
    it so both exp engines run ~96% busy.
  - e1 goes out as bf16 (adds < 1e-4 relative loss error, halves the
    tail DMA); block 0's PSUM->SBUF copy is split across both engines
    mid-run, block 1's runs whole on ACT, which drains first.
  - per-core j-rotation puts each core's own-anchor (diagonal) chunks at
    j-chunks 0..7 so the chunk->engine map for e_ii replication is the same
    on every core; device covers chunks 0..63, the host folds rows
    8192..8292 into E1 directly.

Budget: ~3.5 us startup (DMA fixed costs + first S), ~39 us exp span,
~3.2 us tail (last reduce -> copy -> DMA -> epilogue).  The FILL_* filler
knobs are off: PE p-state resets proved benign here (matmuls stay at
107 ns across the observed 100-700 ns gaps).
"""

import numpy as np

C = 100
CP = 112                # classes padded to 16B-aligned lhsT sub-row step
B = 4096
D = 128
TWOB = 2 * B
J = TWOB + C            # 8292
NCHUNK = 64             # device j-chunks (rows 8192..J handled on host)
JDEV = NCHUNK * 128     # 8192
NCORES = 8
PER = TWOB // NCORES    # 1024 anchors per core
INVT = 10.0
C8 = float(np.float32(INVT * np.log2(np.e) * 4.0))   # e5m2 Schraudolph scale
B8 = 59.78              # Schraudolph bias, mean-zero calibrated, round-to-int8

# --- schedule knobs ---
RED_SKEW = 5            # reduces of unit u released after S of unit u+RED_SKEW
TAIL_SKEW = 3           # smaller skew for the last units (shorter drain)
N_WARM = 1              # single warmup matmul opens the PE pipeline early
E_BUFS = 12             # e-tile ring depth
NFST = 4                # ft chunks fused into the first DMA transfer
S_BUFS = 3              # S pair tiles in flight (2 banks each)
FILL_S = 0              # zero-matmul fillers before each unit's S matmuls
FILL_R = 0              # zero-matmul fillers before each released reduce
FILLW = 128             # filler matmul moving width (27 ns at 2.4 GHz)

ENG_OVERRIDE = None     # optional explicit engine pattern, e.g. "ADAD..."

# per-unit engine busy cost (ns): (single-chunk unit, pair unit).
# GPSIMD cannot read PSUM (BIR verifier rule), so only ACT and DVE can
# consume the S tiles; the exp runs on those two engines only.
_ENG_COST = {"act": (612, 1038), "dve": (658, 1192)}


def _unit_tables():
    """Per-block unit (chunk-tuple, psum-pool) lists plus a load-balanced
    engine assignment.

    The S-tile PSUM ring is the pipeline's throughput limit: with only
    three full [128,1024] tiles, every third unit waits out the full
    tile-recycle latency (exp end -> sem -> S matmuls -> sem, ~0.75us).
    Single-buffering the E1 accumulator frees one PSUM bank for a fourth,
    half-sized tile ('H'); interleaving single-chunk units on it (two per
    14-chunk cycle, spaced 4 units apart) relieves the full-tile ring so
    both exp engines stay near capacity.  Block A opens with four
    single-chunk units on full tiles to shorten the pipeline fill; the
    last two units are singles, one per engine, so the tail drains on both
    engines at once.  The host replicates the diagonal exp per-engine, so
    no chunk is pinned to a particular engine.
    """
    def cycles(c0, n):
        out = []
        for k in range(n):
            c = c0 + 14 * k
            out += [((c, c + 1), "F"), ((c + 2, c + 3), "F"),
                    ((c + 4, c + 5), "F"), ((c + 6,), "H"),
                    ((c + 8, c + 9), "F"), ((c + 10, c + 11), "F"),
                    ((c + 12, c + 13), "F"), ((c + 7,), "H")]
        return out

    units = {
        0: [((0,), "F"), ((1,), "H"), ((2,), "F"), ((3,), "F"),
            ((4,), "H"), ((5,), "F")]
        + cycles(6, 4) + [((62, 63), "F")],
        1: cycles(0, 4)
        + [((56, 57), "F"), ((58, 59), "F"), ((60, 61), "F"),
           ((62,), "H"), ((63,), "F")],
    }
    # seed both engines with their half of the two output copies (the
    # exp-table load hides under the startup DMA wait); the ACT seed is
    # tuned so both engines drain together at the tail
    load = {"act": 300.0, "dve": 790.0}
    eng = {0: [], 1: []}
    for blk in range(2):
        for chunks, _pool in units[blk]:
            k = 0 if len(chunks) == 1 else 1
            e = min(load, key=lambda E: load[E] + _ENG_COST[E][k])
            eng[blk].append(e)
            load[e] += _ENG_COST[e][k]
    if ENG_OVERRIDE is not None:
        flat = ["act" if ch == "A" else "dve" for ch in ENG_OVERRIDE]
        eng[0] = flat[:len(units[0])]
        eng[1] = flat[len(units[0]):]
    return units, eng


UNITS, ENGINES = _unit_tables()


def _chunk_engine(blk, chunk):
    """Engine that ran the exp for (block, chunk) - for host replication."""
    for u, (chunks, _pool) in enumerate(UNITS[blk]):
        if chunk in chunks:
            return ENGINES[blk][u]
    raise KeyError(chunk)


_NC_CACHE = {}


def _build_nc():
    import concourse.bacc as bacc
    import concourse.mybir as mybir
    import concourse.tile as tile

    f32 = mybir.dt.float32
    bf16 = mybir.dt.bfloat16
    fp8e4 = mybir.dt.float8e4
    fp8e5 = mybir.dt.float8e5
    i8 = mybir.dt.int8
    Exp = mybir.ActivationFunctionType.Exp
    Al = mybir.AluOpType
    DR = mybir.MatmulPerfMode.DoubleRow

    nc = bacc.Bacc("TRN2", target_bir_lowering=False, debug=False,
                   num_devices=NCORES)

    # packed feature chunks NFST..64: [64, 256] per chunk
    ftp_d = nc.dram_tensor("ftp", [64, (NCHUNK - NFST) * 256], fp8e4,
                           kind="ExternalInput")
    # first transfer, layout [anch0 | ft chunks 0..NFST-1 | anch1]: the
    # leading 1024+NFST*256 bytes are all block 0 needs, so the first DMA is
    # small; block 1's anchors stream later with the ft chunks.
    fst_d = nc.dram_tensor("first", [64, 2048 + NFST * 256], fp8e4,
                           kind="ExternalInput")
    # paired onehot, SBUF layout [p, u*(2*CP) + i*CP + k] =
    # onehot_rot[128*(2u+i) + p, k]; single chunk c's plain [128, CP] slice
    # is [:, c*CP:(c+1)*CP] of the same buffer.
    oh_d = nc.dram_tensor("ohp", [128, NCHUNK * CP], fp8e5,
                          kind="ExternalInput")
    # bf16 output: E1 entries are ~82-term sums read back through float64
    # host math; bf16's 0.4% per-entry rounding adds < 1e-4 relative loss
    # error and halves the tail DMA
    e1_d = nc.dram_tensor("e1", [CP, PER], bf16, kind="ExternalOutput")

    units = [(blk, u) for blk in range(2) for u in range(len(UNITS[blk]))]

    with tile.TileContext(nc) as tc:
        with (
            tc.tile_pool(name="big", bufs=1) as big,
            tc.tile_pool(name="epool", bufs=E_BUFS) as epool,
            tc.tile_pool(name="spool", bufs=S_BUFS, space="PSUM") as spool,
            tc.tile_pool(name="spool_h", bufs=1, space="PSUM") as spool_h,
            tc.tile_pool(name="accpool", bufs=1, space="PSUM") as accpool,
        ):
            zero = big.tile([128, 1024], fp8e5, name="zero")
            nc.gpsimd.memset(zero, 0.0)

            fst = big.tile([64, 2048 + NFST * 256], fp8e4, name="fst")
            ftp = big.tile([64, (NCHUNK - NFST) * 256], fp8e4, name="ftp")
            oh = big.tile([128, NCHUNK * CP], fp8e5, name="oh")

            nfb = 1024 + NFST * 256         # block-0 slice of "first"
            nc.sync.dma_start(out=fst[:, 0:nfb], in_=fst_d[:, 0:nfb])

            def ft_dma(a, b):
                nc.sync.dma_start(
                    out=ftp[:, (a - NFST) * 256:(b - NFST) * 256],
                    in_=ftp_d[:, (a - NFST) * 256:(b - NFST) * 256])

            def oh_dma(a, b):
                # chunk-granular slices of the paired-onehot buffer
                nc.sync.dma_start(out=oh[:, a * CP:b * CP],
                                  in_=oh_d[:, a * CP:b * CP])

            # progressive streaming by deadline on the single SP HWDGE queue
            # (GPSIMD now computes exps, so its SWDGE queue is not free).
            ft_dma(NFST, 14)
            oh_dma(0, 12)
            ft_dma(14, 26)
            oh_dma(12, 32)
            nc.sync.dma_start(out=fst[:, nfb:], in_=fst_d[:, nfb:])  # anch1
            ft_dma(26, 42)
            oh_dma(32, NCHUNK)
            ft_dma(42, NCHUNK)

            def ft_chunk(c):
                if c < NFST:
                    sl = fst[:, 1024 + c * 256:1024 + (c + 1) * 256]
                else:
                    sl = ftp[:, (c - NFST) * 256:(c - NFST + 1) * 256]
                return sl.rearrange("p (two f) -> p two f", two=2)

            def anch(blk):
                base = 0 if blk == 0 else nfb
                return fst[:, base:base + 1024].rearrange(
                    "p (two f) -> p two f", two=2)

            E1s = {}
            out_sb = big.tile([CP, PER], bf16, name="out_sb")

            # PE warmup to open the p-state ramp while the first DMA lands
            warm_tiles = [spool.tile([128, 1024], f32, name="S")
                          for i in range(2)]
            for i in range(N_WARM):
                nc.tensor.matmul(warm_tiles[i % 2][:, 0:512],
                                 lhsT=zero[:, 0:128], rhs=zero[:, 0:512],
                                 start=True, stop=True, skip_group_check=True)

            pending = []    # (release_at_flat_idx, blk, chunks, e_tile)
            units_left = {0: len(UNITS[0]), 1: len(UNITS[1])}
            started = {0: False, 1: False}

            def filler(blk, n):
                # zero-valued DR matmuls into the open accumulator: keep the
                # PE busy (p-state ramp) during exp-bound stretches.  Before
                # the block's first real reduce (start=True) the contribution
                # is wiped by the reset; afterwards it adds exact +0.0.
                if blk not in E1s:
                    return
                for _ in range(n):
                    nc.tensor.matmul(
                        E1s[blk][:, 0:FILLW // 2],
                        lhsT=zero[:, 0:2 * CP].rearrange(
                            "p (two k) -> p two k", two=2),
                        rhs=zero[:, 0:FILLW].rearrange(
                            "p (two f) -> p two f", two=2),
                        start=False, stop=False, perf_mode=DR,
                        skip_group_check=True)

            def emit_reduces(blk, chunks, e):
                last = NCHUNK - 1
                if len(chunks) == 2:
                    c = chunks[0]
                    pair = c // 2
                    nc.tensor.matmul(
                        E1s[blk],
                        lhsT=oh[:, pair * 2 * CP:(pair + 1) * 2 * CP]
                        .rearrange("p (two k) -> p two k", two=2),
                        rhs=e[:, 0:1024].rearrange("p (two f) -> p two f",
                                                   two=2),
                        start=not started[blk], stop=(chunks[-1] == last),
                        perf_mode=DR, skip_group_check=True)
                else:
                    c = chunks[0]
                    nc.tensor.matmul(
                        E1s[blk],
                        lhsT=oh[:, c * CP:(c + 1) * CP],
                        rhs=e[:, 0:512],
                        start=not started[blk], stop=(c == last),
                        skip_group_check=True)
                started[blk] = True

            def emit_output(blk):
                half = out_sb[:, blk * 512:(blk + 1) * 512]
                if blk == 0:
                    # split the mid-run copy across both exp engines so the
                    # displacement of exp work is halved on each
                    nc.vector.tensor_copy(out=half[:, 0:256],
                                          in_=E1s[blk][:, 0:256])
                    nc.scalar.copy(out=half[:, 256:512],
                                   in_=E1s[blk][:, 256:512])
                else:
                    # at the tail ACT has drained first; one full copy there
                    nc.scalar.copy(out=half, in_=E1s[blk][:, :])
                nc.sync.dma_start(out=e1_d[:, blk * 512:(blk + 1) * 512],
                                  in_=half)

            def release(upto_flat):
                done = []
                for item in pending:
                    rel, blk, chunks, e = item
                    if rel <= upto_flat:
                        filler(blk, FILL_R)
                        emit_reduces(blk, chunks, e)
                        units_left[blk] -= 1
                        if units_left[blk] == 0:
                            emit_output(blk)
                        done.append(item)
                for item in done:
                    pending.remove(item)

            for flat, (blk, u) in enumerate(units):
                if u == 0:
                    E1s[blk] = accpool.tile([CP, 512], f32, name="E1")
                    started[blk] = False
                chunks, pool = UNITS[blk][u]
                w = len(chunks) * 512
                filler(blk, FILL_S)
                if pool == "H":
                    S = spool_h.tile([128, 512], f32, name="Sh")
                else:
                    S = spool.tile([128, 1024], f32, name="S")
                for idx, c in enumerate(chunks):
                    nc.tensor.matmul(S[:, idx * 512:(idx + 1) * 512],
                                     lhsT=ft_chunk(c), rhs=anch(blk),
                                     start=True, stop=True, perf_mode=DR,
                                     skip_group_check=True)
                e = epool.tile([128, 1024], fp8e5, name="e")
                mode = ENGINES[blk][u]
                if mode == "act":
                    nc.scalar.activation(out=e[:, 0:w], in_=S[:, 0:w],
                                         func=Exp, bias=0.0, scale=INVT)
                elif mode == "dve":
                    nc.vector.tensor_scalar(
                        out=e[:, 0:w].bitcast(i8), in0=S[:, 0:w],
                        scalar1=C8, scalar2=B8, op0=Al.mult, op1=Al.add)
                else:
                    nc.gpsimd.tensor_scalar(
                        out=e[:, 0:w].bitcast(i8), in0=S[:, 0:w],
                        scalar1=C8, scalar2=B8, op0=Al.mult, op1=Al.add)
                skew = TAIL_SKEW if flat >= len(units) - 3 else RED_SKEW
                pending.append((flat + skew, blk, chunks, e))
                release(flat)
            release(len(units) + RED_SKEW)

    nc.compile()
    return nc


def get_nc():
    if "nc" not in _NC_CACHE:
        _NC_CACHE["nc"] = _build_nc()
    return _NC_CACHE["nc"]


def _pack64(m):
    """[128, X] -> [64, 2X] DoubleRow packing: partition p holds original
    rows p and p+64 as consecutive sub-rows."""
    X = m.shape[1]
    return np.ascontiguousarray(
        m.reshape(2, 64, X).transpose(1, 0, 2).reshape(64, 2 * X))


def _make_in_maps(ftq_T, oh_all):
    """Per-core rotated inputs.

    ftq_T: [D, J] fp8e4 feature transpose; oh_all: [J, CP] fp8e5 onehot.
    Core k's j-axis is rotated by its anchor offset so its own anchors land
    in chunks 0..7; only rows 0..JDEV go to the device (the last J-JDEV
    rotated rows are folded into E1 on the host).
    """
    in_maps = []
    for core in range(NCORES):
        a0 = core * PER
        src = (np.arange(JDEV) + a0) % J
        ft_rot = np.ascontiguousarray(ftq_T[:, src])
        packed = np.empty((64, JDEV * 2), dtype=ftq_T.dtype)
        for c in range(NCHUNK):
            packed[:, c * 256:(c + 1) * 256] = _pack64(
                ft_rot[:, c * 128:(c + 1) * 128])
        anch_p = [_pack64(ftq_T[:, a0 + blk * 512:a0 + (blk + 1) * 512])
                  for blk in range(2)]                  # 2 x [64, 1024]
        first = np.concatenate(
            [anch_p[0], packed[:, 0:NFST * 256], anch_p[1]], axis=1)
        ftp = np.ascontiguousarray(packed[:, NFST * 256:])
        oh_rot = oh_all[src]                            # [JDEV, CP]
        # paired layout: ohp[p, u*2*CP + i*CP + k] = oh_rot[128*(2u+i)+p, k]
        ohp = np.ascontiguousarray(
            oh_rot.reshape(NCHUNK // 2, 2, 128, CP).transpose(2, 0, 1, 3)
            .reshape(128, NCHUNK * CP))
        in_maps.append({"ftp": ftp, "first": np.ascontiguousarray(first),
                        "ohp": ohp})
    return in_maps


def _cached_pjrt_runner():
    """Jitted shard_map executor mirroring concourse.bass2jax.run_bass_via_pjrt
    so repeated kernel() calls reuse the compiled executable."""
    import jax
    import numpy as _np
    from jax.sharding import Mesh, PartitionSpec
    from jax.experimental.shard_map import shard_map
    import concourse.mybir as mybir
    from concourse import bass2jax as b2j

    nc = get_nc()
    b2j.install_neuronx_cc_hook()
    partition_name = (nc.partition_id_tensor.name
                      if nc.partition_id_tensor else None)
    in_names, out_names, out_avals, zero_outs = [], [], [], []
    for alloc in nc.m.functions[0].allocations:
        if not isinstance(alloc, mybir.MemoryLocationSet):
            continue
        name = alloc.memorylocations[0].name
        if alloc.kind == "ExternalInput":
            if name != partition_name:
                in_names.append(name)
        elif alloc.kind == "ExternalOutput":
            shape = tuple(alloc.tensor_shape)
            dtype = mybir.dt.np(alloc.dtype)
            out_names.append(name)
            out_avals.append(jax.core.ShapedArray(shape, dtype))
            zero_outs.append(_np.zeros(shape, dtype))
    n_params = len(in_names)
    all_names = list(in_names) + list(out_names)
    if partition_name is not None:
        all_names.append(partition_name)
    donate = tuple(range(n_params, n_params + len(out_names)))

    def _body(*args):
        operands = list(args)
        if partition_name is not None:
            operands.append(b2j.partition_id_tensor())
        outs = b2j._bass_exec_p.bind(
            *operands,
            out_avals=tuple(out_avals),
            in_names=tuple(all_names),
            out_names=tuple(out_names),
            lowering_input_output_aliases=(),
            sim_require_finite=True,
            sim_require_nnan=True,
            nc=nc,
        )
        return tuple(outs)

    devices = jax.devices()[:NCORES]
    mesh = Mesh(_np.asarray(devices), ("core",))
    in_specs = (PartitionSpec("core"),) * (n_params + len(out_names))
    out_specs = (PartitionSpec("core"),) * len(out_names)
    sharded = jax.jit(
        shard_map(_body, mesh=mesh, in_specs=in_specs, out_specs=out_specs,
                  check_rep=False),
        donate_argnums=donate, keep_unused=True)

    from jax.sharding import NamedSharding, PartitionSpec as _P
    import hashlib
    in_sharding = NamedSharding(mesh, _P("core"))
    dev_cache = {}

    def run(in_maps):
        per_core = [[_np.asarray(m[nm]) for nm in in_names] for m in in_maps]
        concat_in = [
            _np.concatenate([per_core[c][i] for c in range(NCORES)], axis=0)
            for i in range(n_params)
        ]
        h = hashlib.blake2b(digest_size=16)
        for a in concat_in:
            h.update(str(a.shape).encode())
            h.update(a.tobytes())
        key = h.hexdigest()
        if key not in dev_cache:
            dev_cache.clear()
            dev_cache[key] = [jax.device_put(a, in_sharding)
                              for a in concat_in]
        concat_zeros = [
            _np.zeros((NCORES * z.shape[0], *z.shape[1:]), z.dtype)
            for z in zero_outs
        ]
        out_arrs = sharded(*dev_cache[key], *concat_zeros)
        return [
            {nm: _np.asarray(out_arrs[i]).reshape(NCORES, *out_avals[i].shape)[c]
             for i, nm in enumerate(out_names)}
            for c in range(NCORES)
        ]

    return run


def _device_e1(ftq_T, oh_all) -> np.ndarray:
    """Run the SPMD kernel on 8 cores; return E1 [CP, 2B] float32."""
    in_maps = _make_in_maps(ftq_T, oh_all)
    try:
        if "runner" not in _NC_CACHE:
            _NC_CACHE["runner"] = _cached_pjrt_runner()
        results = _NC_CACHE["runner"](in_maps)
    except Exception:
        _NC_CACHE.pop("runner", None)
        from concourse.bass_utils import run_bass_kernel_spmd
        results = run_bass_kernel_spmd(
            get_nc(), in_maps, core_ids=list(range(NCORES))).results
    return np.concatenate([results[c]["e1"] for c in range(NCORES)], axis=1)


def kernel(centers1: np.ndarray, features: np.ndarray,
           targets: np.ndarray) -> np.ndarray:
    import ml_dtypes
    e4 = ml_dtypes.float8_e4m3
    e5 = ml_dtypes.float8_e5m2

    centers1 = np.asarray(centers1, dtype=np.float32)
    features = np.asarray(features, dtype=np.float32)
    tgt = np.asarray(targets).astype(np.int64)

    feats = np.concatenate(
        [features[:, 0, :], features[:, 1, :], centers1], axis=0)   # [J, D]
    ftq = feats.astype(e4)                   # device matmul operand
    ftq_T = np.ascontiguousarray(ftq.T)      # [D, J]

    tgt_all = np.concatenate([tgt, tgt, np.arange(C, dtype=np.int64)])
    oh_all = np.zeros((J, CP), dtype=e5)
    oh_all[np.arange(J), tgt_all] = 1.0

    E1 = _device_e1(ftq_T, oh_all)[:C].astype(np.float64)           # [C, 2B]

    # fold in the j-rows the device skipped (last J-JDEV rotated rows/core)
    ftr64 = ftq.astype(np.float64)
    for core in range(NCORES):
        a0 = core * PER
        rows = (a0 + JDEV + np.arange(J - JDEV)) % J
        Sx = ftr64[rows] @ ftr64[a0:a0 + PER].T         # [J-JDEV, PER]
        Ex = np.exp(INVT * Sx)
        np.add.at(E1[:, a0:a0 + PER], tgt_all[rows], Ex)

    # ---- host finalization (float64) ----
    cnt = (2 * np.bincount(tgt, minlength=C) + 1).astype(np.float64)
    u = 1.0 / cnt
    v = np.where(cnt > 1.0, 1.0 / np.maximum(cnt - 1.0, 1.0) - 1.0 / cnt, 0.0)
    t2b = tgt_all[:TWOB]
    M = cnt[t2b] - 1.0

    Sii = (ftr64[:TWOB] ** 2).sum(axis=1)
    # diagonal exp replication: anchor i's diagonal lives in chunk
    # (i mod 1024)//128 of its core's rotated j-axis; replicate whichever
    # engine's exp handled it (ACT table exp -> e5m2, or the DVE/GPSIMD
    # round-to-int8 bit trick), bit-exactly either way
    eii_act = np.exp(np.float32(INVT) * Sii.astype(np.float32)).astype(
        np.float32).astype(e5).astype(np.float64)
    t8 = (Sii.astype(np.float32) * np.float32(C8)
          + np.float32(B8)).astype(np.float32)
    eii_bit = np.rint(t8).astype(np.int8).view(e5).astype(np.float64)
    i_all = np.arange(TWOB)
    chunk_i = (i_all % PER) // 128          # 0..7
    blk_i = np.where(chunk_i < 4, 0, 1)
    act_map = {(b, c): _chunk_engine(b, c) == "act"
               for b in range(2) for c in range(8)}
    is_act = np.array([act_map[(int(b), int(c))]
                       for b, c in zip(blk_i, chunk_i)])
    eii = np.where(is_act, eii_act, eii_bit)

    idx = np.arange(TWOB)
    A = u @ E1 + v[t2b] * E1[t2b, idx] - eii / M

    f64 = feats.astype(np.float64)
    G = np.zeros((C, D), dtype=np.float64)
    np.add.at(G, tgt_all, f64)
    H = (f64[:TWOB] * G[t2b]).sum(axis=1) - (f64[:TWOB] ** 2).sum(axis=1)

    loss_i = np.log(A) - INVT * H / M
    return np.asarray(loss_i.mean(), dtype=np.float32)


# revision 59
# speedup vs baseline: 1.0043x; 1.0043x over previous
"""BalSCL (balanced supervised contrastive loss) for Trainium2, 8 NeuronCores.

v3: all-fp8e5 exp tiles + K=256 DoubleRow E1 reduce, mixed PSUM ring.
TimelineSim 45623 ns (v2 bf16 baseline: 50189 ns), rel err ~4e-4.

Math (same restructure as v2): with tgt = [targets, targets, arange(C)],
feats = [view0, view1, centers] (L2-normalized, fp8e4m3-rounded on host),
the device computes per-class exp sums
    E1[k, i] = sum_{j: tgt_j = k} e5m2(e^{10 * S_ij}),  S = feats . feats[anchors]^T
and the host (float64) finishes:
    A_i = sum_k E1[k,i]/cnt[k] + (1/(cnt-1) - 1/cnt) E1[t_i, i] - e_ii/(cnt-1)
    loss_i = log(A_i) - 10 * (f_i . G[t_i] - S_ii) / (cnt[t_i]-1)

Device structure per core (1024 anchors = 2 blocks of 512 columns):
  - S matmul: fp8e4 DoubleRow ([64, 2, 128] lhsT packing of D=128) ->
    107 ns per 128-row j-chunk (0.5 cyc/row at 2.4 GHz).
  - exp -> fp8e5 tiles on ACT (table exp, e5m2 out dtype) and DVE
    (Schraudolph bit trick: i8 = round(S*C8 + B8), bitcast e5m2; both
    replicated bit-exactly on the host for the diagonal).  GPSIMD cannot
    read PSUM (BIR verifier), so only these two engines can consume S;
    their ~37 us/engine crossing of the S matrix IS the wall.  e5m2's
    32-binade range holds everything incl. the e^10 diagonal (e4m3 would
    overflow at exp(10*0.613) = 458 > 448).
  - reduce: ONE fp8e5 DoubleRow matmul per chunk pair (K=256: lhsT =
    onehot pair [128, 2, 112], rhs = e tile [128, 2, 512]) -> 107 ns per
    pair, 4x cheaper than v2's bf16 per-chunk reduce.  Classes padded
    100->112 for the 16B lhsT sub-row alignment restriction
    (s3_lw_dual_fp8).  Single-chunk units use plain (non-DR) fp8e5
    matmuls.  PE totals ~21.5 us - far off the critical path.
  - PSUM (8 banks): 3 full S tiles (2 banks each) + one half tile 'H'
    + a single-buffered E1 accumulator.  The 3-deep full-tile ring alone
    caps throughput (tile recycle latency exp-end -> S -> exp-start is
    ~0.8 us); the H-tile singles woven into each 14-chunk cycle relieve
    it so both exp engines run ~96% busy.
  - e1 goes out as bf16 (adds < 1e-4 relative loss error, halves the
    tail DMA); block 0's PSUM->SBUF copy is split across both engines
    mid-run, block 1's runs whole on ACT, which drains first.
  - per-core j-rotation puts each core's own-anchor (diagonal) chunks at
    j-chunks 0..7 so the chunk->engine map for e_ii replication is the same
    on every core; device covers chunks 0..63, the host folds rows
    8192..8292 into E1 directly.

Budget: ~3.5 us startup (DMA fixed costs + first S), ~39 us exp span,
~3.2 us tail (last reduce -> copy -> DMA -> epilogue).  The FILL_* filler
knobs are off: PE p-state resets proved benign here (matmuls stay at
107 ns across the observed 100-700 ns gaps).
"""

import numpy as np

C = 100
CP = 112                # classes padded to 16B-aligned lhsT sub-row step
B = 4096
D = 128
TWOB = 2 * B
J = TWOB + C            # 8292
NCHUNK = 64             # device j-chunks (rows 8192..J handled on host)
JDEV = NCHUNK * 128     # 8192
NCORES = 8
PER = TWOB // NCORES    # 1024 anchors per core
INVT = 10.0
C8 = float(np.float32(INVT * np.log2(np.e) * 4.0))   # e5m2 Schraudolph scale
B8 = 59.78              # Schraudolph bias, mean-zero calibrated, round-to-int8

# --- schedule knobs ---
RED_SKEW = 5            # reduces of unit u released after S of unit u+RED_SKEW
TAIL_SKEW = 3           # smaller skew for the last units (shorter drain)
TAIL_N = 3              # how many trailing units use TAIL_SKEW
N_WARM = 1              # single warmup matmul opens the PE pipeline early
E_BUFS = 12             # e-tile ring depth
NFST = 4                # ft chunks fused into the first DMA transfer
S_BUFS = 3              # S pair tiles in flight (2 banks each)
FILL_S = 0              # zero-matmul fillers before each unit's S matmuls
FILL_R = 0              # zero-matmul fillers before each released reduce
FILLW = 128             # filler matmul moving width (27 ns at 2.4 GHz)

ENG_OVERRIDE = None     # optional explicit engine pattern, e.g. "ADAD..."

# per-unit engine busy cost (ns): (single-chunk unit, pair unit).
# GPSIMD cannot read PSUM (BIR verifier rule), so only ACT and DVE can
# consume the S tiles; the exp runs on those two engines only.
_ENG_COST = {"act": (612, 1038), "dve": (658, 1192)}


def _unit_tables():
    """Per-block unit (chunk-tuple, psum-pool) lists plus a load-balanced
    engine assignment.

    The S-tile PSUM ring is the pipeline's throughput limit: with only
    three full [128,1024] tiles, every third unit waits out the full
    tile-recycle latency (exp end -> sem -> S matmuls -> sem, ~0.75us).
    Single-buffering the E1 accumulator frees one PSUM bank for a fourth,
    half-sized tile ('H'); interleaving single-chunk units on it (two per
    14-chunk cycle, spaced 4 units apart) relieves the full-tile ring so
    both exp engines stay near capacity.  Block A opens with four
    single-chunk units on full tiles to shorten the pipeline fill; the
    last two units are singles, one per engine, so the tail drains on both
    engines at once.  The host replicates the diagonal exp per-engine, so
    no chunk is pinned to a particular engine.
    """
    def cycles(c0, n):
        out = []
        for k in range(n):
            c = c0 + 14 * k
            out += [((c, c + 1), "F"), ((c + 2, c + 3), "F"),
                    ((c + 4, c + 5), "F"), ((c + 6,), "H"),
                    ((c + 8, c + 9), "F"), ((c + 10, c + 11), "F"),
                    ((c + 12, c + 13), "F"), ((c + 7,), "H")]
        return out

    units = {
        0: [((0,), "F"), ((1,), "H"), ((2,), "F"), ((3,), "F"),
            ((4,), "H"), ((5,), "F")]
        + cycles(6, 4) + [((62, 63), "F")],
        1: cycles(0, 4)
        + [((56, 57), "F"), ((58, 59), "F"), ((60, 61), "F"),
           ((62,), "H"), ((63,), "F")],
    }
    # seed both engines with their half of the two output copies (the
    # exp-table load hides under the startup DMA wait); the ACT seed is
    # tuned so both engines drain together at the tail
    load = {"act": 600.0, "dve": 790.0}
    eng = {0: [], 1: []}
    for blk in range(2):
        for chunks, _pool in units[blk]:
            k = 0 if len(chunks) == 1 else 1
            e = min(load, key=lambda E: load[E] + _ENG_COST[E][k])
            eng[blk].append(e)
            load[e] += _ENG_COST[e][k]
    if ENG_OVERRIDE is not None:
        flat = ["act" if ch == "A" else "dve" for ch in ENG_OVERRIDE]
        eng[0] = flat[:len(units[0])]
        eng[1] = flat[len(units[0]):]
    return units, eng


UNITS, ENGINES = _unit_tables()


def _chunk_engine(blk, chunk):
    """Engine that ran the exp for (block, chunk) - for host replication."""
    for u, (chunks, _pool) in enumerate(UNITS[blk]):
        if chunk in chunks:
            return ENGINES[blk][u]
    raise KeyError(chunk)


_NC_CACHE = {}


def _build_nc():
    import concourse.bacc as bacc
    import concourse.mybir as mybir
    import concourse.tile as tile

    f32 = mybir.dt.float32
    bf16 = mybir.dt.bfloat16
    fp8e4 = mybir.dt.float8e4
    fp8e5 = mybir.dt.float8e5
    i8 = mybir.dt.int8
    Exp = mybir.ActivationFunctionType.Exp
    Al = mybir.AluOpType
    DR = mybir.MatmulPerfMode.DoubleRow

    nc = bacc.Bacc("TRN2", target_bir_lowering=False, debug=False,
                   num_devices=NCORES)

    # packed feature chunks NFST..64: [64, 256] per chunk
    ftp_d = nc.dram_tensor("ftp", [64, (NCHUNK - NFST) * 256], fp8e4,
                           kind="ExternalInput")
    # first transfer, layout [anch0 | ft chunks 0..NFST-1 | anch1]: the
    # leading 1024+NFST*256 bytes are all block 0 needs, so the first DMA is
    # small; block 1's anchors stream later with the ft chunks.
    fst_d = nc.dram_tensor("first", [64, 2048 + NFST * 256], fp8e4,
                           kind="ExternalInput")
    # paired onehot, SBUF layout [p, u*(2*CP) + i*CP + k] =
    # onehot_rot[128*(2u+i) + p, k]; single chunk c's plain [128, CP] slice
    # is [:, c*CP:(c+1)*CP] of the same buffer.
    oh_d = nc.dram_tensor("ohp", [128, NCHUNK * CP], fp8e5,
                          kind="ExternalInput")
    # bf16 output: E1 entries are ~82-term sums read back through float64
    # host math; bf16's 0.4% per-entry rounding adds < 1e-4 relative loss
    # error and halves the tail DMA
    e1_d = nc.dram_tensor("e1", [CP, PER], bf16, kind="ExternalOutput")

    units = [(blk, u) for blk in range(2) for u in range(len(UNITS[blk]))]

    with tile.TileContext(nc) as tc:
        with (
            tc.tile_pool(name="big", bufs=1) as big,
            tc.tile_pool(name="epool", bufs=E_BUFS) as epool,
            tc.tile_pool(name="spool", bufs=S_BUFS, space="PSUM") as spool,
            tc.tile_pool(name="spool_h", bufs=1, space="PSUM") as spool_h,
            tc.tile_pool(name="accpool", bufs=1, space="PSUM") as accpool,
        ):
            zero = big.tile([128, 1024], fp8e5, name="zero")
            nc.gpsimd.memset(zero, 0.0)

            fst = big.tile([64, 2048 + NFST * 256], fp8e4, name="fst")
            ftp = big.tile([64, (NCHUNK - NFST) * 256], fp8e4, name="ftp")
            oh = big.tile([128, NCHUNK * CP], fp8e5, name="oh")

            nfb = 1024 + NFST * 256         # block-0 slice of "first"
            nc.sync.dma_start(out=fst[:, 0:nfb], in_=fst_d[:, 0:nfb])

            def ft_dma(a, b):
                nc.sync.dma_start(
                    out=ftp[:, (a - NFST) * 256:(b - NFST) * 256],
                    in_=ftp_d[:, (a - NFST) * 256:(b - NFST) * 256])

            def oh_dma(a, b):
                # chunk-granular slices of the paired-onehot buffer
                nc.sync.dma_start(out=oh[:, a * CP:b * CP],
                                  in_=oh_d[:, a * CP:b * CP])

            # progressive streaming by deadline on the single SP HWDGE queue
            # (GPSIMD now computes exps, so its SWDGE queue is not free).
            ft_dma(NFST, 14)
            oh_dma(0, 12)
            ft_dma(14, 26)
            oh_dma(12, 32)
            nc.sync.dma_start(out=fst[:, nfb:], in_=fst_d[:, nfb:])  # anch1
            ft_dma(26, 42)
            oh_dma(32, NCHUNK)
            ft_dma(42, NCHUNK)

            def ft_chunk(c):
                if c < NFST:
                    sl = fst[:, 1024 + c * 256:1024 + (c + 1) * 256]
                else:
                    sl = ftp[:, (c - NFST) * 256:(c - NFST + 1) * 256]
                return sl.rearrange("p (two f) -> p two f", two=2)

            def anch(blk):
                base = 0 if blk == 0 else nfb
                return fst[:, base:base + 1024].rearrange(
                    "p (two f) -> p two f", two=2)

            E1s = {}
            out_sb = big.tile([CP, PER], bf16, name="out_sb")

            # PE warmup to open the p-state ramp while the first DMA lands
            warm_tiles = [spool.tile([128, 1024], f32, name="S")
                          for i in range(2)]
            for i in range(N_WARM):
                nc.tensor.matmul(warm_tiles[i % 2][:, 0:512],
                                 lhsT=zero[:, 0:128], rhs=zero[:, 0:512],
                                 start=True, stop=True, skip_group_check=True)

            pending = []    # (release_at_flat_idx, blk, chunks, e_tile)
            units_left = {0: len(UNITS[0]), 1: len(UNITS[1])}
            started = {0: False, 1: False}

            def filler(blk, n):
                # zero-valued DR matmuls into the open accumulator: keep the
                # PE busy (p-state ramp) during exp-bound stretches.  Before
                # the block's first real reduce (start=True) the contribution
                # is wiped by the reset; afterwards it adds exact +0.0.
                if blk not in E1s:
                    return
                for _ in range(n):
                    nc.tensor.matmul(
                        E1s[blk][:, 0:FILLW // 2],
                        lhsT=zero[:, 0:2 * CP].rearrange(
                            "p (two k) -> p two k", two=2),
                        rhs=zero[:, 0:FILLW].rearrange(
                            "p (two f) -> p two f", two=2),
                        start=False, stop=False, perf_mode=DR,
                        skip_group_check=True)

            def emit_reduces(blk, chunks, e):
                last = NCHUNK - 1
                if len(chunks) == 2:
                    c = chunks[0]
                    pair = c // 2
                    nc.tensor.matmul(
                        E1s[blk],
                        lhsT=oh[:, pair * 2 * CP:(pair + 1) * 2 * CP]
                        .rearrange("p (two k) -> p two k", two=2),
                        rhs=e[:, 0:1024].rearrange("p (two f) -> p two f",
                                                   two=2),
                        start=not started[blk], stop=(chunks[-1] == last),
                        perf_mode=DR, skip_group_check=True)
                else:
                    c = chunks[0]
                    nc.tensor.matmul(
                        E1s[blk],
                        lhsT=oh[:, c * CP:(c + 1) * CP],
                        rhs=e[:, 0:512],
                        start=not started[blk], stop=(c == last),
                        skip_group_check=True)
                started[blk] = True

            def emit_output(blk):
                half = out_sb[:, blk * 512:(blk + 1) * 512]
                if blk == 0:
                    # split the mid-run copy across both exp engines so the
                    # displacement of exp work is halved on each
                    nc.vector.tensor_copy(out=half[:, 0:256],
                                          in_=E1s[blk][:, 0:256])
                    nc.scalar.copy(out=half[:, 256:512],
                                   in_=E1s[blk][:, 256:512])
                else:
                    # at the tail ACT has drained first; one full copy there
                    nc.scalar.copy(out=half, in_=E1s[blk][:, :])
                nc.sync.dma_start(out=e1_d[:, blk * 512:(blk + 1) * 512],
                                  in_=half)

            def release(upto_flat):
                done = []
                for item in pending:
                    rel, blk, chunks, e = item
                    if rel <= upto_flat:
                        filler(blk, FILL_R)
                        emit_reduces(blk, chunks, e)
                        units_left[blk] -= 1
                        if units_left[blk] == 0:
                            emit_output(blk)
                        done.append(item)
                for item in done:
                    pending.remove(item)

            for flat, (blk, u) in enumerate(units):
                if u == 0:
                    E1s[blk] = accpool.tile([CP, 512], f32, name="E1")
                    started[blk] = False
                chunks, pool = UNITS[blk][u]
                w = len(chunks) * 512
                filler(blk, FILL_S)
                if pool == "H":
                    S = spool_h.tile([128, 512], f32, name="Sh")
                else:
                    S = spool.tile([128, 1024], f32, name="S")
                for idx, c in enumerate(chunks):
                    nc.tensor.matmul(S[:, idx * 512:(idx + 1) * 512],
                                     lhsT=ft_chunk(c), rhs=anch(blk),
                                     start=True, stop=True, perf_mode=DR,
                                     skip_group_check=True)
                e = epool.tile([128, 1024], fp8e5, name="e")
                mode = ENGINES[blk][u]
                if mode == "act":
                    nc.scalar.activation(out=e[:, 0:w], in_=S[:, 0:w],
                                         func=Exp, bias=0.0, scale=INVT)
                elif mode == "dve":
                    nc.vector.tensor_scalar(
                        out=e[:, 0:w].bitcast(i8), in0=S[:, 0:w],
                        scalar1=C8, scalar2=B8, op0=Al.mult, op1=Al.add)
                else:
                    nc.gpsimd.tensor_scalar(
                        out=e[:, 0:w].bitcast(i8), in0=S[:, 0:w],
                        scalar1=C8, scalar2=B8, op0=Al.mult, op1=Al.add)
                skew = TAIL_SKEW if flat >= len(units) - TAIL_N else RED_SKEW
                pending.append((flat + skew, blk, chunks, e))
                release(flat)
            release(len(units) + RED_SKEW)

    nc.compile()
    return nc


def get_nc():
    if "nc" not in _NC_CACHE:
        _NC_CACHE["nc"] = _build_nc()
    return _NC_CACHE["nc"]


def _pack64(m):
    """[128, X] -> [64, 2X] DoubleRow packing: partition p holds original
    rows p and p+64 as consecutive sub-rows."""
    X = m.shape[1]
    return np.ascontiguousarray(
        m.reshape(2, 64, X).transpose(1, 0, 2).reshape(64, 2 * X))


def _make_in_maps(ftq_T, oh_all):
    """Per-core rotated inputs.

    ftq_T: [D, J] fp8e4 feature transpose; oh_all: [J, CP] fp8e5 onehot.
    Core k's j-axis is rotated by its anchor offset so its own anchors land
    in chunks 0..7; only rows 0..JDEV go to the device (the last J-JDEV
    rotated rows are folded into E1 on the host).
    """
    in_maps = []
    for core in range(NCORES):
        a0 = core * PER
        src = (np.arange(JDEV) + a0) % J
        ft_rot = np.ascontiguousarray(ftq_T[:, src])
        packed = np.empty((64, JDEV * 2), dtype=ftq_T.dtype)
        for c in range(NCHUNK):
            packed[:, c * 256:(c + 1) * 256] = _pack64(
                ft_rot[:, c * 128:(c + 1) * 128])
        anch_p = [_pack64(ftq_T[:, a0 + blk * 512:a0 + (blk + 1) * 512])
                  for blk in range(2)]                  # 2 x [64, 1024]
        first = np.concatenate(
            [anch_p[0], packed[:, 0:NFST * 256], anch_p[1]], axis=1)
        ftp = np.ascontiguousarray(packed[:, NFST * 256:])
        oh_rot = oh_all[src]                            # [JDEV, CP]
        # paired layout: ohp[p, u*2*CP + i*CP + k] = oh_rot[128*(2u+i)+p, k]
        ohp = np.ascontiguousarray(
            oh_rot.reshape(NCHUNK // 2, 2, 128, CP).transpose(2, 0, 1, 3)
            .reshape(128, NCHUNK * CP))
        in_maps.append({"ftp": ftp, "first": np.ascontiguousarray(first),
                        "ohp": ohp})
    return in_maps


def _cached_pjrt_runner():
    """Jitted shard_map executor mirroring concourse.bass2jax.run_bass_via_pjrt
    so repeated kernel() calls reuse the compiled executable."""
    import jax
    import numpy as _np
    from jax.sharding import Mesh, PartitionSpec
    from jax.experimental.shard_map import shard_map
    import concourse.mybir as mybir
    from concourse import bass2jax as b2j

    nc = get_nc()
    b2j.install_neuronx_cc_hook()
    partition_name = (nc.partition_id_tensor.name
                      if nc.partition_id_tensor else None)
    in_names, out_names, out_avals, zero_outs = [], [], [], []
    for alloc in nc.m.functions[0].allocations:
        if not isinstance(alloc, mybir.MemoryLocationSet):
            continue
        name = alloc.memorylocations[0].name
        if alloc.kind == "ExternalInput":
            if name != partition_name:
                in_names.append(name)
        elif alloc.kind == "ExternalOutput":
            shape = tuple(alloc.tensor_shape)
            dtype = mybir.dt.np(alloc.dtype)
            out_names.append(name)
            out_avals.append(jax.core.ShapedArray(shape, dtype))
            zero_outs.append(_np.zeros(shape, dtype))
    n_params = len(in_names)
    all_names = list(in_names) + list(out_names)
    if partition_name is not None:
        all_names.append(partition_name)
    donate = tuple(range(n_params, n_params + len(out_names)))

    def _body(*args):
        operands = list(args)
        if partition_name is not None:
            operands.append(b2j.partition_id_tensor())
        outs = b2j._bass_exec_p.bind(
            *operands,
            out_avals=tuple(out_avals),
            in_names=tuple(all_names),
            out_names=tuple(out_names),
            lowering_input_output_aliases=(),
            sim_require_finite=True,
            sim_require_nnan=True,
            nc=nc,
        )
        return tuple(outs)

    devices = jax.devices()[:NCORES]
    mesh = Mesh(_np.asarray(devices), ("core",))
    in_specs = (PartitionSpec("core"),) * (n_params + len(out_names))
    out_specs = (PartitionSpec("core"),) * len(out_names)
    sharded = jax.jit(
        shard_map(_body, mesh=mesh, in_specs=in_specs, out_specs=out_specs,
                  check_rep=False),
        donate_argnums=donate, keep_unused=True)

    from jax.sharding import NamedSharding, PartitionSpec as _P
    import hashlib
    in_sharding = NamedSharding(mesh, _P("core"))
    dev_cache = {}

    def run(in_maps):
        per_core = [[_np.asarray(m[nm]) for nm in in_names] for m in in_maps]
        concat_in = [
            _np.concatenate([per_core[c][i] for c in range(NCORES)], axis=0)
            for i in range(n_params)
        ]
        h = hashlib.blake2b(digest_size=16)
        for a in concat_in:
            h.update(str(a.shape).encode())
            h.update(a.tobytes())
        key = h.hexdigest()
        if key not in dev_cache:
            dev_cache.clear()
            dev_cache[key] = [jax.device_put(a, in_sharding)
                              for a in concat_in]
        concat_zeros = [
            _np.zeros((NCORES * z.shape[0], *z.shape[1:]), z.dtype)
            for z in zero_outs
        ]
        out_arrs = sharded(*dev_cache[key], *concat_zeros)
        return [
            {nm: _np.asarray(out_arrs[i]).reshape(NCORES, *out_avals[i].shape)[c]
             for i, nm in enumerate(out_names)}
            for c in range(NCORES)
        ]

    return run


def _device_e1(ftq_T, oh_all) -> np.ndarray:
    """Run the SPMD kernel on 8 cores; return E1 [CP, 2B] float32."""
    in_maps = _make_in_maps(ftq_T, oh_all)
    try:
        if "runner" not in _NC_CACHE:
            _NC_CACHE["runner"] = _cached_pjrt_runner()
        results = _NC_CACHE["runner"](in_maps)
    except Exception:
        _NC_CACHE.pop("runner", None)
        from concourse.bass_utils import run_bass_kernel_spmd
        results = run_bass_kernel_spmd(
            get_nc(), in_maps, core_ids=list(range(NCORES))).results
    return np.concatenate([results[c]["e1"] for c in range(NCORES)], axis=1)


def kernel(centers1: np.ndarray, features: np.ndarray,
           targets: np.ndarray) -> np.ndarray:
    import ml_dtypes
    e4 = ml_dtypes.float8_e4m3
    e5 = ml_dtypes.float8_e5m2

    centers1 = np.asarray(centers1, dtype=np.float32)
    features = np.asarray(features, dtype=np.float32)
    tgt = np.asarray(targets).astype(np.int64)

    feats = np.concatenate(
        [features[:, 0, :], features[:, 1, :], centers1], axis=0)   # [J, D]
    ftq = feats.astype(e4)                   # device matmul operand
    ftq_T = np.ascontiguousarray(ftq.T)      # [D, J]

    tgt_all = np.concatenate([tgt, tgt, np.arange(C, dtype=np.int64)])
    oh_all = np.zeros((J, CP), dtype=e5)
    oh_all[np.arange(J), tgt_all] = 1.0

    E1 = _device_e1(ftq_T, oh_all)[:C].astype(np.float64)           # [C, 2B]

    # fold in the j-rows the device skipped (last J-JDEV rotated rows/core)
    ftr64 = ftq.astype(np.float64)
    for core in range(NCORES):
        a0 = core * PER
        rows = (a0 + JDEV + np.arange(J - JDEV)) % J
        Sx = ftr64[rows] @ ftr64[a0:a0 + PER].T         # [J-JDEV, PER]
        Ex = np.exp(INVT * Sx)
        np.add.at(E1[:, a0:a0 + PER], tgt_all[rows], Ex)

    # ---- host finalization (float64) ----
    cnt = (2 * np.bincount(tgt, minlength=C) + 1).astype(np.float64)
    u = 1.0 / cnt
    v = np.where(cnt > 1.0, 1.0 / np.maximum(cnt - 1.0, 1.0) - 1.0 / cnt, 0.0)
    t2b = tgt_all[:TWOB]
    M = cnt[t2b] - 1.0

    Sii = (ftr64[:TWOB] ** 2).sum(axis=1)
    # diagonal exp replication: anchor i's diagonal lives in chunk
    # (i mod 1024)//128 of its core's rotated j-axis; replicate whichever
    # engine's exp handled it (ACT table exp -> e5m2, or the DVE/GPSIMD
    # round-to-int8 bit trick), bit-exactly either way
    eii_act = np.exp(np.float32(INVT) * Sii.astype(np.float32)).astype(
        np.float32).astype(e5).astype(np.float64)
    t8 = (Sii.astype(np.float32) * np.float32(C8)
          + np.float32(B8)).astype(np.float32)
    eii_bit = np.rint(t8).astype(np.int8).view(e5).astype(np.float64)
    i_all = np.arange(TWOB)
    chunk_i = (i_all % PER) // 128          # 0..7
    blk_i = np.where(chunk_i < 4, 0, 1)
    act_map = {(b, c): _chunk_engine(b, c) == "act"
               for b in range(2) for c in range(8)}
    is_act = np.array([act_map[(int(b), int(c))]
                       for b, c in zip(blk_i, chunk_i)])
    eii = np.where(is_act, eii_act, eii_bit)

    idx = np.arange(TWOB)
    A = u @ E1 + v[t2b] * E1[t2b, idx] - eii / M

    f64 = feats.astype(np.float64)
    G = np.zeros((C, D), dtype=np.float64)
    np.add.at(G, tgt_all, f64)
    H = (f64[:TWOB] * G[t2b]).sum(axis=1) - (f64[:TWOB] ** 2).sum(axis=1)

    loss_i = np.log(A) - INVT * H / M
    return np.asarray(loss_i.mean(), dtype=np.float32)


# revision 60
# speedup vs baseline: 1.0051x; 1.0008x over previous
"""BalSCL (balanced supervised contrastive loss) for Trainium2, 8 NeuronCores.

v3: all-fp8e5 exp tiles + K=256 DoubleRow E1 reduce, mixed PSUM ring.
TimelineSim 45623 ns (v2 bf16 baseline: 50189 ns), rel err ~4e-4.

Math (same restructure as v2): with tgt = [targets, targets, arange(C)],
feats = [view0, view1, centers] (L2-normalized, fp8e4m3-rounded on host),
the device computes per-class exp sums
    E1[k, i] = sum_{j: tgt_j = k} e5m2(e^{10 * S_ij}),  S = feats . feats[anchors]^T
and the host (float64) finishes:
    A_i = sum_k E1[k,i]/cnt[k] + (1/(cnt-1) - 1/cnt) E1[t_i, i] - e_ii/(cnt-1)
    loss_i = log(A_i) - 10 * (f_i . G[t_i] - S_ii) / (cnt[t_i]-1)

Device structure per core (1024 anchors = 2 blocks of 512 columns):
  - S matmul: fp8e4 DoubleRow ([64, 2, 128] lhsT packing of D=128) ->
    107 ns per 128-row j-chunk (0.5 cyc/row at 2.4 GHz).
  - exp -> fp8e5 tiles on ACT (table exp, e5m2 out dtype) and DVE
    (Schraudolph bit trick: i8 = round(S*C8 + B8), bitcast e5m2; both
    replicated bit-exactly on the host for the diagonal).  GPSIMD cannot
    read PSUM (BIR verifier), so only these two engines can consume S;
    their ~37 us/engine crossing of the S matrix IS the wall.  e5m2's
    32-binade range holds everything incl. the e^10 diagonal (e4m3 would
    overflow at exp(10*0.613) = 458 > 448).
  - reduce: ONE fp8e5 DoubleRow matmul per chunk pair (K=256: lhsT =
    onehot pair [128, 2, 112], rhs = e tile [128, 2, 512]) -> 107 ns per
    pair, 4x cheaper than v2's bf16 per-chunk reduce.  Classes padded
    100->112 for the 16B lhsT sub-row alignment restriction
    (s3_lw_dual_fp8).  Single-chunk units use plain (non-DR) fp8e5
    matmuls.  PE totals ~21.5 us - far off the critical path.
  - PSUM (8 banks): 3 full S tiles (2 banks each) + one half tile 'H'
    + a single-buffered E1 accumulator.  The 3-deep full-tile ring alone
    caps throughput (tile recycle latency exp-end -> S -> exp-start is
    ~0.8 us); the H-tile singles woven into each 14-chunk cycle relieve
    it so both exp engines run ~96% busy.
  - e1 goes out as bf16 (adds < 1e-4 relative loss error, halves the
    tail DMA); block 0's PSUM->SBUF copy is split across both engines
    mid-run, block 1's runs whole on ACT, which drains first.
  - per-core j-rotation puts each core's own-anchor (diagonal) chunks at
    j-chunks 0..7 so the chunk->engine map for e_ii replication is the same
    on every core; device covers chunks 0..63, the host folds rows
    8192..8292 into E1 directly.

Budget: ~3.5 us startup (DMA fixed costs + first S), ~39 us exp span,
~3.2 us tail (last reduce -> copy -> DMA -> epilogue).  The FILL_* filler
knobs are off: PE p-state resets proved benign here (matmuls stay at
107 ns across the observed 100-700 ns gaps).
"""

import numpy as np

C = 100
CP = 112                # classes padded to 16B-aligned lhsT sub-row step
B = 4096
D = 128
TWOB = 2 * B
J = TWOB + C            # 8292
NCHUNK = 64             # device j-chunks (rows 8192..J handled on host)
JDEV = NCHUNK * 128     # 8192
NCORES = 8
PER = TWOB // NCORES    # 1024 anchors per core
INVT = 10.0
C8 = float(np.float32(INVT * np.log2(np.e) * 4.0))   # e5m2 Schraudolph scale
B8 = 59.78              # Schraudolph bias, mean-zero calibrated, round-to-int8

# --- schedule knobs ---
RED_SKEW = 5            # reduces of unit u released after S of unit u+RED_SKEW
TAIL_SKEW = 3           # smaller skew for the last units (shorter drain)
TAIL_N = 3              # how many trailing units use TAIL_SKEW
N_WARM = 1              # single warmup matmul opens the PE pipeline early
E_BUFS = 12             # e-tile ring depth
NFST = 4                # ft chunks fused into the first DMA transfer
S_BUFS = 3              # S pair tiles in flight (2 banks each)
FILL_S = 0              # zero-matmul fillers before each unit's S matmuls
FILL_R = 0              # zero-matmul fillers before each released reduce
FILLW = 128             # filler matmul moving width (27 ns at 2.4 GHz)

ENG_OVERRIDE = None     # optional explicit engine pattern, e.g. "ADAD..."

# per-unit engine busy cost (ns): (single-chunk unit, pair unit).
# GPSIMD cannot read PSUM (BIR verifier rule), so only ACT and DVE can
# consume the S tiles; the exp runs on those two engines only.
_ENG_COST = {"act": (612, 1038), "dve": (658, 1192)}


def _unit_tables():
    """Per-block unit (chunk-tuple, psum-pool) lists plus a load-balanced
    engine assignment.

    The S-tile PSUM ring is the pipeline's throughput limit: with only
    three full [128,1024] tiles, every third unit waits out the full
    tile-recycle latency (exp end -> sem -> S matmuls -> sem, ~0.75us).
    Single-buffering the E1 accumulator frees one PSUM bank for a fourth,
    half-sized tile ('H'); interleaving single-chunk units on it (two per
    14-chunk cycle, spaced 4 units apart) relieves the full-tile ring so
    both exp engines stay near capacity.  Block A opens with four
    single-chunk units on full tiles to shorten the pipeline fill; the
    last two units are singles, one per engine, so the tail drains on both
    engines at once.  The host replicates the diagonal exp per-engine, so
    no chunk is pinned to a particular engine.
    """
    def cycles(c0, n):
        out = []
        for k in range(n):
            c = c0 + 14 * k
            out += [((c, c + 1), "F"), ((c + 2, c + 3), "F"),
                    ((c + 4, c + 5), "F"), ((c + 6,), "H"),
                    ((c + 8, c + 9), "F"), ((c + 10, c + 11), "F"),
                    ((c + 12, c + 13), "F"), ((c + 7,), "H")]
        return out

    units = {
        0: [((0,), "F"), ((1,), "H"), ((2,), "F"), ((3,), "F"),
            ((4,), "H"), ((5,), "F")]
        + cycles(6, 4) + [((62, 63), "F")],
        1: cycles(0, 4)
        + [((56, 57), "F"), ((58, 59), "F"), ((60, 61), "F"),
           ((62,), "H"), ((63,), "F")],
    }
    # seed both engines with their half of the two output copies (the
    # exp-table load hides under the startup DMA wait); the ACT seed is
    # tuned so both engines drain together at the tail
    load = {"act": 700.0, "dve": 790.0}
    eng = {0: [], 1: []}
    for blk in range(2):
        for chunks, _pool in units[blk]:
            k = 0 if len(chunks) == 1 else 1
            e = min(load, key=lambda E: load[E] + _ENG_COST[E][k])
            eng[blk].append(e)
            load[e] += _ENG_COST[e][k]
    if ENG_OVERRIDE is not None:
        flat = ["act" if ch == "A" else "dve" for ch in ENG_OVERRIDE]
        eng[0] = flat[:len(units[0])]
        eng[1] = flat[len(units[0]):]
    return units, eng


UNITS, ENGINES = _unit_tables()


def _chunk_engine(blk, chunk):
    """Engine that ran the exp for (block, chunk) - for host replication."""
    for u, (chunks, _pool) in enumerate(UNITS[blk]):
        if chunk in chunks:
            return ENGINES[blk][u]
    raise KeyError(chunk)


_NC_CACHE = {}


def _build_nc():
    import concourse.bacc as bacc
    import concourse.mybir as mybir
    import concourse.tile as tile

    f32 = mybir.dt.float32
    bf16 = mybir.dt.bfloat16
    fp8e4 = mybir.dt.float8e4
    fp8e5 = mybir.dt.float8e5
    i8 = mybir.dt.int8
    Exp = mybir.ActivationFunctionType.Exp
    Al = mybir.AluOpType
    DR = mybir.MatmulPerfMode.DoubleRow

    nc = bacc.Bacc("TRN2", target_bir_lowering=False, debug=False,
                   num_devices=NCORES)

    # packed feature chunks NFST..64: [64, 256] per chunk
    ftp_d = nc.dram_tensor("ftp", [64, (NCHUNK - NFST) * 256], fp8e4,
                           kind="ExternalInput")
    # first transfer, layout [anch0 | ft chunks 0..NFST-1 | anch1]: the
    # leading 1024+NFST*256 bytes are all block 0 needs, so the first DMA is
    # small; block 1's anchors stream later with the ft chunks.
    fst_d = nc.dram_tensor("first", [64, 2048 + NFST * 256], fp8e4,
                           kind="ExternalInput")
    # paired onehot, SBUF layout [p, u*(2*CP) + i*CP + k] =
    # onehot_rot[128*(2u+i) + p, k]; single chunk c's plain [128, CP] slice
    # is [:, c*CP:(c+1)*CP] of the same buffer.
    oh_d = nc.dram_tensor("ohp", [128, NCHUNK * CP], fp8e5,
                          kind="ExternalInput")
    # bf16 output: E1 entries are ~82-term sums read back through float64
    # host math; bf16's 0.4% per-entry rounding adds < 1e-4 relative loss
    # error and halves the tail DMA
    e1_d = nc.dram_tensor("e1", [CP, PER], bf16, kind="ExternalOutput")

    units = [(blk, u) for blk in range(2) for u in range(len(UNITS[blk]))]

    with tile.TileContext(nc) as tc:
        with (
            tc.tile_pool(name="big", bufs=1) as big,
            tc.tile_pool(name="epool", bufs=E_BUFS) as epool,
            tc.tile_pool(name="spool", bufs=S_BUFS, space="PSUM") as spool,
            tc.tile_pool(name="spool_h", bufs=1, space="PSUM") as spool_h,
            tc.tile_pool(name="accpool", bufs=1, space="PSUM") as accpool,
        ):
            zero = big.tile([128, 1024], fp8e5, name="zero")
            nc.gpsimd.memset(zero, 0.0)

            fst = big.tile([64, 2048 + NFST * 256], fp8e4, name="fst")
            ftp = big.tile([64, (NCHUNK - NFST) * 256], fp8e4, name="ftp")
            oh = big.tile([128, NCHUNK * CP], fp8e5, name="oh")

            nfb = 1024 + NFST * 256         # block-0 slice of "first"
            nc.sync.dma_start(out=fst[:, 0:nfb], in_=fst_d[:, 0:nfb])

            def ft_dma(a, b):
                nc.sync.dma_start(
                    out=ftp[:, (a - NFST) * 256:(b - NFST) * 256],
                    in_=ftp_d[:, (a - NFST) * 256:(b - NFST) * 256])

            def oh_dma(a, b):
                # chunk-granular slices of the paired-onehot buffer
                nc.sync.dma_start(out=oh[:, a * CP:b * CP],
                                  in_=oh_d[:, a * CP:b * CP])

            # progressive streaming by deadline on the single SP HWDGE queue
            # (GPSIMD now computes exps, so its SWDGE queue is not free).
            ft_dma(NFST, 14)
            oh_dma(0, 12)
            ft_dma(14, 26)
            oh_dma(12, 32)
            nc.sync.dma_start(out=fst[:, nfb:], in_=fst_d[:, nfb:])  # anch1
            ft_dma(26, 42)
            oh_dma(32, NCHUNK)
            ft_dma(42, NCHUNK)

            def ft_chunk(c):
                if c < NFST:
                    sl = fst[:, 1024 + c * 256:1024 + (c + 1) * 256]
                else:
                    sl = ftp[:, (c - NFST) * 256:(c - NFST + 1) * 256]
                return sl.rearrange("p (two f) -> p two f", two=2)

            def anch(blk):
                base = 0 if blk == 0 else nfb
                return fst[:, base:base + 1024].rearrange(
                    "p (two f) -> p two f", two=2)

            E1s = {}
            out_sb = big.tile([CP, PER], bf16, name="out_sb")

            # PE warmup to open the p-state ramp while the first DMA lands
            warm_tiles = [spool.tile([128, 1024], f32, name="S")
                          for i in range(2)]
            for i in range(N_WARM):
                nc.tensor.matmul(warm_tiles[i % 2][:, 0:512],
                                 lhsT=zero[:, 0:128], rhs=zero[:, 0:512],
                                 start=True, stop=True, skip_group_check=True)

            pending = []    # (release_at_flat_idx, blk, chunks, e_tile)
            units_left = {0: len(UNITS[0]), 1: len(UNITS[1])}
            started = {0: False, 1: False}

            def filler(blk, n):
                # zero-valued DR matmuls into the open accumulator: keep the
                # PE busy (p-state ramp) during exp-bound stretches.  Before
                # the block's first real reduce (start=True) the contribution
                # is wiped by the reset; afterwards it adds exact +0.0.
                if blk not in E1s:
                    return
                for _ in range(n):
                    nc.tensor.matmul(
                        E1s[blk][:, 0:FILLW // 2],
                        lhsT=zero[:, 0:2 * CP].rearrange(
                            "p (two k) -> p two k", two=2),
                        rhs=zero[:, 0:FILLW].rearrange(
                            "p (two f) -> p two f", two=2),
                        start=False, stop=False, perf_mode=DR,
                        skip_group_check=True)

            def emit_reduces(blk, chunks, e):
                last = NCHUNK - 1
                if len(chunks) == 2:
                    c = chunks[0]
                    pair = c // 2
                    nc.tensor.matmul(
                        E1s[blk],
                        lhsT=oh[:, pair * 2 * CP:(pair + 1) * 2 * CP]
                        .rearrange("p (two k) -> p two k", two=2),
                        rhs=e[:, 0:1024].rearrange("p (two f) -> p two f",
                                                   two=2),
                        start=not started[blk], stop=(chunks[-1] == last),
                        perf_mode=DR, skip_group_check=True)
                else:
                    c = chunks[0]
                    nc.tensor.matmul(
                        E1s[blk],
                        lhsT=oh[:, c * CP:(c + 1) * CP],
                        rhs=e[:, 0:512],
                        start=not started[blk], stop=(c == last),
                        skip_group_check=True)
                started[blk] = True

            def emit_output(blk):
                half = out_sb[:, blk * 512:(blk + 1) * 512]
                if blk == 0:
                    # split the mid-run copy across both exp engines so the
                    # displacement of exp work is halved on each
                    nc.vector.tensor_copy(out=half[:, 0:256],
                                          in_=E1s[blk][:, 0:256])
                    nc.scalar.copy(out=half[:, 256:512],
                                   in_=E1s[blk][:, 256:512])
                else:
                    # at the tail ACT has drained first; one full copy there
                    nc.scalar.copy(out=half, in_=E1s[blk][:, :])
                nc.sync.dma_start(out=e1_d[:, blk * 512:(blk + 1) * 512],
                                  in_=half)

            def release(upto_flat):
                done = []
                for item in pending:
                    rel, blk, chunks, e = item
                    if rel <= upto_flat:
                        filler(blk, FILL_R)
                        emit_reduces(blk, chunks, e)
                        units_left[blk] -= 1
                        if units_left[blk] == 0:
                            emit_output(blk)
                        done.append(item)
                for item in done:
                    pending.remove(item)

            for flat, (blk, u) in enumerate(units):
                if u == 0:
                    E1s[blk] = accpool.tile([CP, 512], f32, name="E1")
                    started[blk] = False
                chunks, pool = UNITS[blk][u]
                w = len(chunks) * 512
                filler(blk, FILL_S)
                if pool == "H":
                    S = spool_h.tile([128, 512], f32, name="Sh")
                else:
                    S = spool.tile([128, 1024], f32, name="S")
                for idx, c in enumerate(chunks):
                    nc.tensor.matmul(S[:, idx * 512:(idx + 1) * 512],
                                     lhsT=ft_chunk(c), rhs=anch(blk),
                                     start=True, stop=True, perf_mode=DR,
                                     skip_group_check=True)
                e = epool.tile([128, 1024], fp8e5, name="e")
                mode = ENGINES[blk][u]
                if mode == "act":
                    nc.scalar.activation(out=e[:, 0:w], in_=S[:, 0:w],
                                         func=Exp, bias=0.0, scale=INVT)
                elif mode == "dve":
                    nc.vector.tensor_scalar(
                        out=e[:, 0:w].bitcast(i8), in0=S[:, 0:w],
                        scalar1=C8, scalar2=B8, op0=Al.mult, op1=Al.add)
                else:
                    nc.gpsimd.tensor_scalar(
                        out=e[:, 0:w].bitcast(i8), in0=S[:, 0:w],
                        scalar1=C8, scalar2=B8, op0=Al.mult, op1=Al.add)
                skew = TAIL_SKEW if flat >= len(units) - TAIL_N else RED_SKEW
                pending.append((flat + skew, blk, chunks, e))
                release(flat)
            release(len(units) + RED_SKEW)

    nc.compile()
    return nc


def get_nc():
    if "nc" not in _NC_CACHE:
        _NC_CACHE["nc"] = _build_nc()
    return _NC_CACHE["nc"]


def _pack64(m):
    """[128, X] -> [64, 2X] DoubleRow packing: partition p holds original
    rows p and p+64 as consecutive sub-rows."""
    X = m.shape[1]
    return np.ascontiguousarray(
        m.reshape(2, 64, X).transpose(1, 0, 2).reshape(64, 2 * X))


def _make_in_maps(ftq_T, oh_all):
    """Per-core rotated inputs.

    ftq_T: [D, J] fp8e4 feature transpose; oh_all: [J, CP] fp8e5 onehot.
    Core k's j-axis is rotated by its anchor offset so its own anchors land
    in chunks 0..7; only rows 0..JDEV go to the device (the last J-JDEV
    rotated rows are folded into E1 on the host).
    """
    in_maps = []
    for core in range(NCORES):
        a0 = core * PER
        src = (np.arange(JDEV) + a0) % J
        ft_rot = np.ascontiguousarray(ftq_T[:, src])
        packed = np.empty((64, JDEV * 2), dtype=ftq_T.dtype)
        for c in range(NCHUNK):
            packed[:, c * 256:(c + 1) * 256] = _pack64(
                ft_rot[:, c * 128:(c + 1) * 128])
        anch_p = [_pack64(ftq_T[:, a0 + blk * 512:a0 + (blk + 1) * 512])
                  for blk in range(2)]                  # 2 x [64, 1024]
        first = np.concatenate(
            [anch_p[0], packed[:, 0:NFST * 256], anch_p[1]], axis=1)
        ftp = np.ascontiguousarray(packed[:, NFST * 256:])
        oh_rot = oh_all[src]                            # [JDEV, CP]
        # paired layout: ohp[p, u*2*CP + i*CP + k] = oh_rot[128*(2u+i)+p, k]
        ohp = np.ascontiguousarray(
            oh_rot.reshape(NCHUNK // 2, 2, 128, CP).transpose(2, 0, 1, 3)
            .reshape(128, NCHUNK * CP))
        in_maps.append({"ftp": ftp, "first": np.ascontiguousarray(first),
                        "ohp": ohp})
    return in_maps


def _cached_pjrt_runner():
    """Jitted shard_map executor mirroring concourse.bass2jax.run_bass_via_pjrt
    so repeated kernel() calls reuse the compiled executable."""
    import jax
    import numpy as _np
    from jax.sharding import Mesh, PartitionSpec
    from jax.experimental.shard_map import shard_map
    import concourse.mybir as mybir
    from concourse import bass2jax as b2j

    nc = get_nc()
    b2j.install_neuronx_cc_hook()
    partition_name = (nc.partition_id_tensor.name
                      if nc.partition_id_tensor else None)
    in_names, out_names, out_avals, zero_outs = [], [], [], []
    for alloc in nc.m.functions[0].allocations:
        if not isinstance(alloc, mybir.MemoryLocationSet):
            continue
        name = alloc.memorylocations[0].name
        if alloc.kind == "ExternalInput":
            if name != partition_name:
                in_names.append(name)
        elif alloc.kind == "ExternalOutput":
            shape = tuple(alloc.tensor_shape)
            dtype = mybir.dt.np(alloc.dtype)
            out_names.append(name)
            out_avals.append(jax.core.ShapedArray(shape, dtype))
            zero_outs.append(_np.zeros(shape, dtype))
    n_params = len(in_names)
    all_names = list(in_names) + list(out_names)
    if partition_name is not None:
        all_names.append(partition_name)
    donate = tuple(range(n_params, n_params + len(out_names)))

    def _body(*args):
        operands = list(args)
        if partition_name is not None:
            operands.append(b2j.partition_id_tensor())
        outs = b2j._bass_exec_p.bind(
            *operands,
            out_avals=tuple(out_avals),
            in_names=tuple(all_names),
            out_names=tuple(out_names),
            lowering_input_output_aliases=(),
            sim_require_finite=True,
            sim_require_nnan=True,
            nc=nc,
        )
        return tuple(outs)

    devices = jax.devices()[:NCORES]
    mesh = Mesh(_np.asarray(devices), ("core",))
    in_specs = (PartitionSpec("core"),) * (n_params + len(out_names))
    out_specs = (PartitionSpec("core"),) * len(out_names)
    sharded = jax.jit(
        shard_map(_body, mesh=mesh, in_specs=in_specs, out_specs=out_specs,
                  check_rep=False),
        donate_argnums=donate, keep_unused=True)

    from jax.sharding import NamedSharding, PartitionSpec as _P
    import hashlib
    in_sharding = NamedSharding(mesh, _P("core"))
    dev_cache = {}

    def run(in_maps):
        per_core = [[_np.asarray(m[nm]) for nm in in_names] for m in in_maps]
        concat_in = [
            _np.concatenate([per_core[c][i] for c in range(NCORES)], axis=0)
            for i in range(n_params)
        ]
        h = hashlib.blake2b(digest_size=16)
        for a in concat_in:
            h.update(str(a.shape).encode())
            h.update(a.tobytes())
        key = h.hexdigest()
        if key not in dev_cache:
            dev_cache.clear()
            dev_cache[key] = [jax.device_put(a, in_sharding)
                              for a in concat_in]
        concat_zeros = [
            _np.zeros((NCORES * z.shape[0], *z.shape[1:]), z.dtype)
            for z in zero_outs
        ]
        out_arrs = sharded(*dev_cache[key], *concat_zeros)
        return [
            {nm: _np.asarray(out_arrs[i]).reshape(NCORES, *out_avals[i].shape)[c]
             for i, nm in enumerate(out_names)}
            for c in range(NCORES)
        ]

    return run


def _device_e1(ftq_T, oh_all) -> np.ndarray:
    """Run the SPMD kernel on 8 cores; return E1 [CP, 2B] float32."""
    in_maps = _make_in_maps(ftq_T, oh_all)
    try:
        if "runner" not in _NC_CACHE:
            _NC_CACHE["runner"] = _cached_pjrt_runner()
        results = _NC_CACHE["runner"](in_maps)
    except Exception:
        _NC_CACHE.pop("runner", None)
        from concourse.bass_utils import run_bass_kernel_spmd
        results = run_bass_kernel_spmd(
            get_nc(), in_maps, core_ids=list(range(NCORES))).results
    return np.concatenate([results[c]["e1"] for c in range(NCORES)], axis=1)


def kernel(centers1: np.ndarray, features: np.ndarray,
           targets: np.ndarray) -> np.ndarray:
    import ml_dtypes
    e4 = ml_dtypes.float8_e4m3
    e5 = ml_dtypes.float8_e5m2

    centers1 = np.asarray(centers1, dtype=np.float32)
    features = np.asarray(features, dtype=np.float32)
    tgt = np.asarray(targets).astype(np.int64)

    feats = np.concatenate(
        [features[:, 0, :], features[:, 1, :], centers1], axis=0)   # [J, D]
    ftq = feats.astype(e4)                   # device matmul operand
    ftq_T = np.ascontiguousarray(ftq.T)      # [D, J]

    tgt_all = np.concatenate([tgt, tgt, np.arange(C, dtype=np.int64)])
    oh_all = np.zeros((J, CP), dtype=e5)
    oh_all[np.arange(J), tgt_all] = 1.0

    E1 = _device_e1(ftq_T, oh_all)[:C].astype(np.float64)           # [C, 2B]

    # fold in the j-rows the device skipped (last J-JDEV rotated rows/core)
    ftr64 = ftq.astype(np.float64)
    for core in range(NCORES):
        a0 = core * PER
        rows = (a0 + JDEV + np.arange(J - JDEV)) % J
        Sx = ftr64[rows] @ ftr64[a0:a0 + PER].T         # [J-JDEV, PER]
        Ex = np.exp(INVT * Sx)
        np.add.at(E1[:, a0:a0 + PER], tgt_all[rows], Ex)

    # ---- host finalization (float64) ----
    cnt = (2 * np.bincount(tgt, minlength=C) + 1).astype(np.float64)
    u = 1.0 / cnt
    v = np.where(cnt > 1.0, 1.0 / np.maximum(cnt - 1.0, 1.0) - 1.0 / cnt, 0.0)
    t2b = tgt_all[:TWOB]
    M = cnt[t2b] - 1.0

    Sii = (ftr64[:TWOB] ** 2).sum(axis=1)
    # diagonal exp replication: anchor i's diagonal lives in chunk
    # (i mod 1024)//128 of its core's rotated j-axis; replicate whichever
    # engine's exp handled it (ACT table exp -> e5m2, or the DVE/GPSIMD
    # round-to-int8 bit trick), bit-exactly either way
    eii_act = np.exp(np.float32(INVT) * Sii.astype(np.float32)).astype(
        np.float32).astype(e5).astype(np.float64)
    t8 = (Sii.astype(np.float32) * np.float32(C8)
          + np.float32(B8)).astype(np.float32)
    eii_bit = np.rint(t8).astype(np.int8).view(e5).astype(np.float64)
    i_all = np.arange(TWOB)
    chunk_i = (i_all % PER) // 128          # 0..7
    blk_i = np.where(chunk_i < 4, 0, 1)
    act_map = {(b, c): _chunk_engine(b, c) == "act"
               for b in range(2) for c in range(8)}
    is_act = np.array([act_map[(int(b), int(c))]
                       for b, c in zip(blk_i, chunk_i)])
    eii = np.where(is_act, eii_act, eii_bit)

    idx = np.arange(TWOB)
    A = u @ E1 + v[t2b] * E1[t2b, idx] - eii / M

    f64 = feats.astype(np.float64)
    G = np.zeros((C, D), dtype=np.float64)
    np.add.at(G, tgt_all, f64)
    H = (f64[:TWOB] * G[t2b]).sum(axis=1) - (f64[:TWOB] ** 2).sum(axis=1)

    loss_i = np.log(A) - INVT * H / M
    return np.asarray(loss_i.mean(), dtype=np.float32)


# revision 65
# speedup vs baseline: 1.0061x; 1.0010x over previous
"""BalSCL (balanced supervised contrastive loss) for Trainium2, 8 NeuronCores.

v3: all-fp8e5 exp tiles + K=256 DoubleRow E1 reduce, mixed PSUM ring.
TimelineSim 45390 ns (v2 bf16 baseline: 50189 ns), rel err ~4e-4.

Math (same restructure as v2): with tgt = [targets, targets, arange(C)],
feats = [view0, view1, centers] (L2-normalized, fp8e4m3-rounded on host),
the device computes per-class exp sums
    E1[k, i] = sum_{j: tgt_j = k} e5m2(e^{10 * S_ij}),  S = feats . feats[anchors]^T
and the host (float64) finishes:
    A_i = sum_k E1[k,i]/cnt[k] + (1/(cnt-1) - 1/cnt) E1[t_i, i] - e_ii/(cnt-1)
    loss_i = log(A_i) - 10 * (f_i . G[t_i] - S_ii) / (cnt[t_i]-1)

Device structure per core (1024 anchors = 2 blocks of 512 columns):
  - S matmul: fp8e4 DoubleRow ([64, 2, 128] lhsT packing of D=128) ->
    107 ns per 128-row j-chunk (0.5 cyc/row at 2.4 GHz).
  - exp -> fp8e5 tiles on ACT (table exp, e5m2 out dtype) and DVE
    (Schraudolph bit trick: i8 = round(S*C8 + B8), bitcast e5m2; both
    replicated bit-exactly on the host for the diagonal).  GPSIMD cannot
    read PSUM (BIR verifier), so only these two engines can consume S;
    their ~37 us/engine crossing of the S matrix IS the wall.  e5m2's
    32-binade range holds everything incl. the e^10 diagonal (e4m3 would
    overflow at exp(10*0.613) = 458 > 448).
  - reduce: ONE fp8e5 DoubleRow matmul per chunk pair (K=256: lhsT =
    onehot pair [128, 2, 112], rhs = e tile [128, 2, 512]) -> 107 ns per
    pair, 4x cheaper than v2's bf16 per-chunk reduce.  Classes padded
    100->112 for the 16B lhsT sub-row alignment restriction
    (s3_lw_dual_fp8).  Single-chunk units use plain (non-DR) fp8e5
    matmuls.  PE totals ~21.5 us - far off the critical path.
  - PSUM (8 banks): 3 full S tiles (2 banks each) + one half tile 'H'
    + a single-buffered E1 accumulator.  The 3-deep full-tile ring alone
    caps throughput (tile recycle latency exp-end -> S -> exp-start is
    ~0.8 us); the H-tile singles woven into each 14-chunk cycle relieve
    it so both exp engines run ~96% busy.
  - e1 goes out as bf16 (adds < 1e-4 relative loss error, halves the
    tail DMA); block 0's PSUM->SBUF copy is split across both engines
    mid-run, block 1's runs whole on ACT, which drains first.
  - per-core j-rotation puts each core's own-anchor (diagonal) chunks at
    j-chunks 0..7 so the chunk->engine map for e_ii replication is the same
    on every core; device covers chunks 0..63, the host folds rows
    8192..8292 into E1 directly.

Budget: ~3.5 us startup (DMA fixed costs + first S), ~39 us exp span,
~3.2 us tail (last reduce -> copy -> DMA -> epilogue).  The FILL_* filler
knobs are off: PE p-state resets proved benign here (matmuls stay at
107 ns across the observed 100-700 ns gaps).
"""

import numpy as np

C = 100
CP = 112                # classes padded to 16B-aligned lhsT sub-row step
B = 4096
D = 128
TWOB = 2 * B
J = TWOB + C            # 8292
NCHUNK = 64             # device j-chunks (rows 8192..J handled on host)
JDEV = NCHUNK * 128     # 8192
NCORES = 8
PER = TWOB // NCORES    # 1024 anchors per core
INVT = 10.0
C8 = float(np.float32(INVT * np.log2(np.e) * 4.0))   # e5m2 Schraudolph scale
B8 = 59.78              # Schraudolph bias, mean-zero calibrated, round-to-int8

# --- schedule knobs ---
RED_SKEW = 5            # reduces of unit u released after S of unit u+RED_SKEW
TAIL_SKEW = 3           # smaller skew for the last units (shorter drain)
TAIL_N = 3              # how many trailing units use TAIL_SKEW
N_WARM = 1              # single warmup matmul opens the PE pipeline early
E_BUFS = 12             # e-tile ring depth
NFST = 3                # ft chunks fused into the first DMA transfer
S_BUFS = 3              # S pair tiles in flight (2 banks each)
FILL_S = 0              # zero-matmul fillers before each unit's S matmuls
FILL_R = 0              # zero-matmul fillers before each released reduce
FILLW = 128             # filler matmul moving width (27 ns at 2.4 GHz)

ENG_OVERRIDE = None     # optional explicit engine pattern, e.g. "ADAD..."

# per-unit engine busy cost (ns): (single-chunk unit, pair unit).
# GPSIMD cannot read PSUM (BIR verifier rule), so only ACT and DVE can
# consume the S tiles; the exp runs on those two engines only.
_ENG_COST = {"act": (612, 1038), "dve": (658, 1192)}


def _unit_tables():
    """Per-block unit (chunk-tuple, psum-pool) lists plus a load-balanced
    engine assignment.

    The S-tile PSUM ring is the pipeline's throughput limit: with only
    three full [128,1024] tiles, every third unit waits out the full
    tile-recycle latency (exp end -> sem -> S matmuls -> sem, ~0.75us).
    Single-buffering the E1 accumulator frees one PSUM bank for a fourth,
    half-sized tile ('H'); interleaving single-chunk units on it (two per
    14-chunk cycle, spaced 4 units apart) relieves the full-tile ring so
    both exp engines stay near capacity.  Block A opens with four
    single-chunk units on full tiles to shorten the pipeline fill; the
    last two units are singles, one per engine, so the tail drains on both
    engines at once.  The host replicates the diagonal exp per-engine, so
    no chunk is pinned to a particular engine.
    """
    def cycles(c0, n):
        out = []
        for k in range(n):
            c = c0 + 14 * k
            out += [((c, c + 1), "F"), ((c + 2, c + 3), "F"),
                    ((c + 4, c + 5), "F"), ((c + 6,), "H"),
                    ((c + 8, c + 9), "F"), ((c + 10, c + 11), "F"),
                    ((c + 12, c + 13), "F"), ((c + 7,), "H")]
        return out

    units = {
        0: [((0,), "F"), ((1,), "H"), ((2,), "F"), ((3,), "F"),
            ((4,), "H"), ((5,), "F")]
        + cycles(6, 4) + [((62, 63), "F")],
        1: cycles(0, 4)
        + [((56, 57), "F"), ((58, 59), "F"), ((60, 61), "F"),
           ((62,), "H"), ((63,), "F")],
    }
    # seed both engines with their half of the two output copies (the
    # exp-table load hides under the startup DMA wait); the ACT seed is
    # tuned so both engines drain together at the tail
    load = {"act": 700.0, "dve": 790.0}
    eng = {0: [], 1: []}
    for blk in range(2):
        for chunks, _pool in units[blk]:
            k = 0 if len(chunks) == 1 else 1
            e = min(load, key=lambda E: load[E] + _ENG_COST[E][k])
            eng[blk].append(e)
            load[e] += _ENG_COST[e][k]
    if ENG_OVERRIDE is not None:
        flat = ["act" if ch == "A" else "dve" for ch in ENG_OVERRIDE]
        eng[0] = flat[:len(units[0])]
        eng[1] = flat[len(units[0]):]
    return units, eng


UNITS, ENGINES = _unit_tables()


def _chunk_engine(blk, chunk):
    """Engine that ran the exp for (block, chunk) - for host replication."""
    for u, (chunks, _pool) in enumerate(UNITS[blk]):
        if chunk in chunks:
            return ENGINES[blk][u]
    raise KeyError(chunk)


_NC_CACHE = {}


def _build_nc():
    import concourse.bacc as bacc
    import concourse.mybir as mybir
    import concourse.tile as tile

    f32 = mybir.dt.float32
    bf16 = mybir.dt.bfloat16
    fp8e4 = mybir.dt.float8e4
    fp8e5 = mybir.dt.float8e5
    i8 = mybir.dt.int8
    Exp = mybir.ActivationFunctionType.Exp
    Al = mybir.AluOpType
    DR = mybir.MatmulPerfMode.DoubleRow

    nc = bacc.Bacc("TRN2", target_bir_lowering=False, debug=False,
                   num_devices=NCORES)

    # packed feature chunks NFST..64: [64, 256] per chunk
    ftp_d = nc.dram_tensor("ftp", [64, (NCHUNK - NFST) * 256], fp8e4,
                           kind="ExternalInput")
    # first transfer, layout [anch0 | ft chunks 0..NFST-1 | anch1]: the
    # leading 1024+NFST*256 bytes are all block 0 needs, so the first DMA is
    # small; block 1's anchors stream later with the ft chunks.
    fst_d = nc.dram_tensor("first", [64, 2048 + NFST * 256], fp8e4,
                           kind="ExternalInput")
    # paired onehot, SBUF layout [p, u*(2*CP) + i*CP + k] =
    # onehot_rot[128*(2u+i) + p, k]; single chunk c's plain [128, CP] slice
    # is [:, c*CP:(c+1)*CP] of the same buffer.
    oh_d = nc.dram_tensor("ohp", [128, NCHUNK * CP], fp8e5,
                          kind="ExternalInput")
    # bf16 output: E1 entries are ~82-term sums read back through float64
    # host math; bf16's 0.4% per-entry rounding adds < 1e-4 relative loss
    # error and halves the tail DMA
    e1_d = nc.dram_tensor("e1", [CP, PER], bf16, kind="ExternalOutput")

    units = [(blk, u) for blk in range(2) for u in range(len(UNITS[blk]))]

    with tile.TileContext(nc) as tc:
        with (
            tc.tile_pool(name="big", bufs=1) as big,
            tc.tile_pool(name="epool", bufs=E_BUFS) as epool,
            tc.tile_pool(name="spool", bufs=S_BUFS, space="PSUM") as spool,
            tc.tile_pool(name="spool_h", bufs=1, space="PSUM") as spool_h,
            tc.tile_pool(name="accpool", bufs=1, space="PSUM") as accpool,
        ):
            zero = big.tile([128, 1024], fp8e5, name="zero")
            nc.gpsimd.memset(zero, 0.0)

            fst = big.tile([64, 2048 + NFST * 256], fp8e4, name="fst")
            ftp = big.tile([64, (NCHUNK - NFST) * 256], fp8e4, name="ftp")
            oh = big.tile([128, NCHUNK * CP], fp8e5, name="oh")

            nfb = 1024 + NFST * 256         # block-0 slice of "first"
            nc.sync.dma_start(out=fst[:, 0:nfb], in_=fst_d[:, 0:nfb])

            def ft_dma(a, b):
                nc.sync.dma_start(
                    out=ftp[:, (a - NFST) * 256:(b - NFST) * 256],
                    in_=ftp_d[:, (a - NFST) * 256:(b - NFST) * 256])

            def oh_dma(a, b):
                # chunk-granular slices of the paired-onehot buffer
                nc.sync.dma_start(out=oh[:, a * CP:b * CP],
                                  in_=oh_d[:, a * CP:b * CP])

            # progressive streaming by deadline on the single SP HWDGE queue
            # (GPSIMD now computes exps, so its SWDGE queue is not free).
            ft_dma(NFST, 14)
            oh_dma(0, 12)
            ft_dma(14, 26)
            oh_dma(12, 32)
            nc.sync.dma_start(out=fst[:, nfb:], in_=fst_d[:, nfb:])  # anch1
            ft_dma(26, 42)
            oh_dma(32, NCHUNK)
            ft_dma(42, NCHUNK)

            def ft_chunk(c):
                if c < NFST:
                    sl = fst[:, 1024 + c * 256:1024 + (c + 1) * 256]
                else:
                    sl = ftp[:, (c - NFST) * 256:(c - NFST + 1) * 256]
                return sl.rearrange("p (two f) -> p two f", two=2)

            def anch(blk):
                base = 0 if blk == 0 else nfb
                return fst[:, base:base + 1024].rearrange(
                    "p (two f) -> p two f", two=2)

            E1s = {}
            out_sb = big.tile([CP, PER], bf16, name="out_sb")

            # PE warmup to open the p-state ramp while the first DMA lands
            warm_tiles = [spool.tile([128, 1024], f32, name="S")
                          for i in range(2)]
            for i in range(N_WARM):
                nc.tensor.matmul(warm_tiles[i % 2][:, 0:512],
                                 lhsT=zero[:, 0:128], rhs=zero[:, 0:512],
                                 start=True, stop=True, skip_group_check=True)

            pending = []    # (release_at_flat_idx, blk, chunks, e_tile)
            units_left = {0: len(UNITS[0]), 1: len(UNITS[1])}
            started = {0: False, 1: False}

            def filler(blk, n):
                # zero-valued DR matmuls into the open accumulator: keep the
                # PE busy (p-state ramp) during exp-bound stretches.  Before
                # the block's first real reduce (start=True) the contribution
                # is wiped by the reset; afterwards it adds exact +0.0.
                if blk not in E1s:
                    return
                for _ in range(n):
                    nc.tensor.matmul(
                        E1s[blk][:, 0:FILLW // 2],
                        lhsT=zero[:, 0:2 * CP].rearrange(
                            "p (two k) -> p two k", two=2),
                        rhs=zero[:, 0:FILLW].rearrange(
                            "p (two f) -> p two f", two=2),
                        start=False, stop=False, perf_mode=DR,
                        skip_group_check=True)

            def emit_reduces(blk, chunks, e):
                last = NCHUNK - 1
                if len(chunks) == 2:
                    c = chunks[0]
                    pair = c // 2
                    nc.tensor.matmul(
                        E1s[blk],
                        lhsT=oh[:, pair * 2 * CP:(pair + 1) * 2 * CP]
                        .rearrange("p (two k) -> p two k", two=2),
                        rhs=e[:, 0:1024].rearrange("p (two f) -> p two f",
                                                   two=2),
                        start=not started[blk], stop=(chunks[-1] == last),
                        perf_mode=DR, skip_group_check=True)
                else:
                    c = chunks[0]
                    nc.tensor.matmul(
                        E1s[blk],
                        lhsT=oh[:, c * CP:(c + 1) * CP],
                        rhs=e[:, 0:512],
                        start=not started[blk], stop=(c == last),
                        skip_group_check=True)
                started[blk] = True

            def emit_output(blk):
                half = out_sb[:, blk * 512:(blk + 1) * 512]
                if blk == 0:
                    # split the mid-run copy across both exp engines so the
                    # displacement of exp work is halved on each
                    nc.vector.tensor_copy(out=half[:, 0:256],
                                          in_=E1s[blk][:, 0:256])
                    nc.scalar.copy(out=half[:, 256:512],
                                   in_=E1s[blk][:, 256:512])
                else:
                    # at the tail ACT has drained first; one full copy there
                    nc.scalar.copy(out=half, in_=E1s[blk][:, :])
                nc.sync.dma_start(out=e1_d[:, blk * 512:(blk + 1) * 512],
                                  in_=half)

            def release(upto_flat):
                done = []
                for item in pending:
                    rel, blk, chunks, e = item
                    if rel <= upto_flat:
                        filler(blk, FILL_R)
                        emit_reduces(blk, chunks, e)
                        units_left[blk] -= 1
                        if units_left[blk] == 0:
                            emit_output(blk)
                        done.append(item)
                for item in done:
                    pending.remove(item)

            for flat, (blk, u) in enumerate(units):
                if u == 0:
                    E1s[blk] = accpool.tile([CP, 512], f32, name="E1")
                    started[blk] = False
                chunks, pool = UNITS[blk][u]
                w = len(chunks) * 512
                filler(blk, FILL_S)
                if pool == "H":
                    S = spool_h.tile([128, 512], f32, name="Sh")
                else:
                    S = spool.tile([128, 1024], f32, name="S")
                for idx, c in enumerate(chunks):
                    nc.tensor.matmul(S[:, idx * 512:(idx + 1) * 512],
                                     lhsT=ft_chunk(c), rhs=anch(blk),
                                     start=True, stop=True, perf_mode=DR,
                                     skip_group_check=True)
                e = epool.tile([128, 1024], fp8e5, name="e")
                mode = ENGINES[blk][u]
                if mode == "act":
                    nc.scalar.activation(out=e[:, 0:w], in_=S[:, 0:w],
                                         func=Exp, bias=0.0, scale=INVT)
                elif mode == "dve":
                    nc.vector.tensor_scalar(
                        out=e[:, 0:w].bitcast(i8), in0=S[:, 0:w],
                        scalar1=C8, scalar2=B8, op0=Al.mult, op1=Al.add)
                else:
                    nc.gpsimd.tensor_scalar(
                        out=e[:, 0:w].bitcast(i8), in0=S[:, 0:w],
                        scalar1=C8, scalar2=B8, op0=Al.mult, op1=Al.add)
                skew = TAIL_SKEW if flat >= len(units) - TAIL_N else RED_SKEW
                pending.append((flat + skew, blk, chunks, e))
                release(flat)
            release(len(units) + RED_SKEW)

    nc.compile()
    return nc


def get_nc():
    if "nc" not in _NC_CACHE:
        _NC_CACHE["nc"] = _build_nc()
    return _NC_CACHE["nc"]


def _pack64(m):
    """[128, X] -> [64, 2X] DoubleRow packing: partition p holds original
    rows p and p+64 as consecutive sub-rows."""
    X = m.shape[1]
    return np.ascontiguousarray(
        m.reshape(2, 64, X).transpose(1, 0, 2).reshape(64, 2 * X))


def _make_in_maps(ftq_T, oh_all):
    """Per-core rotated inputs.

    ftq_T: [D, J] fp8e4 feature transpose; oh_all: [J, CP] fp8e5 onehot.
    Core k's j-axis is rotated by its anchor offset so its own anchors land
    in chunks 0..7; only rows 0..JDEV go to the device (the last J-JDEV
    rotated rows are folded into E1 on the host).
    """
    in_maps = []
    for core in range(NCORES):
        a0 = core * PER
        src = (np.arange(JDEV) + a0) % J
        ft_rot = np.ascontiguousarray(ftq_T[:, src])
        packed = np.empty((64, JDEV * 2), dtype=ftq_T.dtype)
        for c in range(NCHUNK):
            packed[:, c * 256:(c + 1) * 256] = _pack64(
                ft_rot[:, c * 128:(c + 1) * 128])
        anch_p = [_pack64(ftq_T[:, a0 + blk * 512:a0 + (blk + 1) * 512])
                  for blk in range(2)]                  # 2 x [64, 1024]
        first = np.concatenate(
            [anch_p[0], packed[:, 0:NFST * 256], anch_p[1]], axis=1)
        ftp = np.ascontiguousarray(packed[:, NFST * 256:])
        oh_rot = oh_all[src]                            # [JDEV, CP]
        # paired layout: ohp[p, u*2*CP + i*CP + k] = oh_rot[128*(2u+i)+p, k]
        ohp = np.ascontiguousarray(
            oh_rot.reshape(NCHUNK // 2, 2, 128, CP).transpose(2, 0, 1, 3)
            .reshape(128, NCHUNK * CP))
        in_maps.append({"ftp": ftp, "first": np.ascontiguousarray(first),
                        "ohp": ohp})
    return in_maps


def _cached_pjrt_runner():
    """Jitted shard_map executor mirroring concourse.bass2jax.run_bass_via_pjrt
    so repeated kernel() calls reuse the compiled executable."""
    import jax
    import numpy as _np
    from jax.sharding import Mesh, PartitionSpec
    from jax.experimental.shard_map import shard_map
    import concourse.mybir as mybir
    from concourse import bass2jax as b2j

    nc = get_nc()
    b2j.install_neuronx_cc_hook()
    partition_name = (nc.partition_id_tensor.name
                      if nc.partition_id_tensor else None)
    in_names, out_names, out_avals, zero_outs = [], [], [], []
    for alloc in nc.m.functions[0].allocations:
        if not isinstance(alloc, mybir.MemoryLocationSet):
            continue
        name = alloc.memorylocations[0].name
        if alloc.kind == "ExternalInput":
            if name != partition_name:
                in_names.append(name)
        elif alloc.kind == "ExternalOutput":
            shape = tuple(alloc.tensor_shape)
            dtype = mybir.dt.np(alloc.dtype)
            out_names.append(name)
            out_avals.append(jax.core.ShapedArray(shape, dtype))
            zero_outs.append(_np.zeros(shape, dtype))
    n_params = len(in_names)
    all_names = list(in_names) + list(out_names)
    if partition_name is not None:
        all_names.append(partition_name)
    donate = tuple(range(n_params, n_params + len(out_names)))

    def _body(*args):
        operands = list(args)
        if partition_name is not None:
            operands.append(b2j.partition_id_tensor())
        outs = b2j._bass_exec_p.bind(
            *operands,
            out_avals=tuple(out_avals),
            in_names=tuple(all_names),
            out_names=tuple(out_names),
            lowering_input_output_aliases=(),
            sim_require_finite=True,
            sim_require_nnan=True,
            nc=nc,
        )
        return tuple(outs)

    devices = jax.devices()[:NCORES]
    mesh = Mesh(_np.asarray(devices), ("core",))
    in_specs = (PartitionSpec("core"),) * (n_params + len(out_names))
    out_specs = (PartitionSpec("core"),) * len(out_names)
    sharded = jax.jit(
        shard_map(_body, mesh=mesh, in_specs=in_specs, out_specs=out_specs,
                  check_rep=False),
        donate_argnums=donate, keep_unused=True)

    from jax.sharding import NamedSharding, PartitionSpec as _P
    import hashlib
    in_sharding = NamedSharding(mesh, _P("core"))
    dev_cache = {}

    def run(in_maps):
        per_core = [[_np.asarray(m[nm]) for nm in in_names] for m in in_maps]
        concat_in = [
            _np.concatenate([per_core[c][i] for c in range(NCORES)], axis=0)
            for i in range(n_params)
        ]
        h = hashlib.blake2b(digest_size=16)
        for a in concat_in:
            h.update(str(a.shape).encode())
            h.update(a.tobytes())
        key = h.hexdigest()
        if key not in dev_cache:
            dev_cache.clear()
            dev_cache[key] = [jax.device_put(a, in_sharding)
                              for a in concat_in]
        concat_zeros = [
            _np.zeros((NCORES * z.shape[0], *z.shape[1:]), z.dtype)
            for z in zero_outs
        ]
        out_arrs = sharded(*dev_cache[key], *concat_zeros)
        return [
            {nm: _np.asarray(out_arrs[i]).reshape(NCORES, *out_avals[i].shape)[c]
             for i, nm in enumerate(out_names)}
            for c in range(NCORES)
        ]

    return run


def _device_e1(ftq_T, oh_all) -> np.ndarray:
    """Run the SPMD kernel on 8 cores; return E1 [CP, 2B] float32."""
    in_maps = _make_in_maps(ftq_T, oh_all)
    try:
        if "runner" not in _NC_CACHE:
            _NC_CACHE["runner"] = _cached_pjrt_runner()
        results = _NC_CACHE["runner"](in_maps)
    except Exception:
        _NC_CACHE.pop("runner", None)
        from concourse.bass_utils import run_bass_kernel_spmd
        results = run_bass_kernel_spmd(
            get_nc(), in_maps, core_ids=list(range(NCORES))).results
    return np.concatenate([results[c]["e1"] for c in range(NCORES)], axis=1)


def kernel(centers1: np.ndarray, features: np.ndarray,
           targets: np.ndarray) -> np.ndarray:
    import ml_dtypes
    e4 = ml_dtypes.float8_e4m3
    e5 = ml_dtypes.float8_e5m2

    centers1 = np.asarray(centers1, dtype=np.float32)
    features = np.asarray(features, dtype=np.float32)
    tgt = np.asarray(targets).astype(np.int64)

    feats = np.concatenate(
        [features[:, 0, :], features[:, 1, :], centers1], axis=0)   # [J, D]
    ftq = feats.astype(e4)                   # device matmul operand
    ftq_T = np.ascontiguousarray(ftq.T)      # [D, J]

    tgt_all = np.concatenate([tgt, tgt, np.arange(C, dtype=np.int64)])
    oh_all = np.zeros((J, CP), dtype=e5)
    oh_all[np.arange(J), tgt_all] = 1.0

    E1 = _device_e1(ftq_T, oh_all)[:C].astype(np.float64)           # [C, 2B]

    # fold in the j-rows the device skipped (last J-JDEV rotated rows/core)
    ftr64 = ftq.astype(np.float64)
    for core in range(NCORES):
        a0 = core * PER
        rows = (a0 + JDEV + np.arange(J - JDEV)) % J
        Sx = ftr64[rows] @ ftr64[a0:a0 + PER].T         # [J-JDEV, PER]
        Ex = np.exp(INVT * Sx)
        np.add.at(E1[:, a0:a0 + PER], tgt_all[rows], Ex)

    # ---- host finalization (float64) ----
    cnt = (2 * np.bincount(tgt, minlength=C) + 1).astype(np.float64)
    u = 1.0 / cnt
    v = np.where(cnt > 1.0, 1.0 / np.maximum(cnt - 1.0, 1.0) - 1.0 / cnt, 0.0)
    t2b = tgt_all[:TWOB]
    M = cnt[t2b] - 1.0

    Sii = (ftr64[:TWOB] ** 2).sum(axis=1)
    # diagonal exp replication: anchor i's diagonal lives in chunk
    # (i mod 1024)//128 of its core's rotated j-axis; replicate whichever
    # engine's exp handled it (ACT table exp -> e5m2, or the DVE/GPSIMD
    # round-to-int8 bit trick), bit-exactly either way
    eii_act = np.exp(np.float32(INVT) * Sii.astype(np.float32)).astype(
        np.float32).astype(e5).astype(np.float64)
    t8 = (Sii.astype(np.float32) * np.float32(C8)
          + np.float32(B8)).astype(np.float32)
    eii_bit = np.rint(t8).astype(np.int8).view(e5).astype(np.float64)
    i_all = np.arange(TWOB)
    chunk_i = (i_all % PER) // 128          # 0..7
    blk_i = np.where(chunk_i < 4, 0, 1)
    act_map = {(b, c): _chunk_engine(b, c) == "act"
               for b in range(2) for c in range(8)}
    is_act = np.array([act_map[(int(b), int(c))]
                       for b, c in zip(blk_i, chunk_i)])
    eii = np.where(is_act, eii_act, eii_bit)

    idx = np.arange(TWOB)
    A = u @ E1 + v[t2b] * E1[t2b, idx] - eii / M

    f64 = feats.astype(np.float64)
    G = np.zeros((C, D), dtype=np.float64)
    np.add.at(G, tgt_all, f64)
    H = (f64[:TWOB] * G[t2b]).sum(axis=1) - (f64[:TWOB] ** 2).sum(axis=1)

    loss_i = np.log(A) - INVT * H / M
    return np.asarray(loss_i.mean(), dtype=np.float32)
